# revision 13
# baseline (speedup 1.0000x reference)
"""Trainium2 Bass kernel for nn_Attention_25383256719981.

Dense transformer attention block:
  qkv = x @ W_qkv.T ; rotary(q,k,v) ; causal+padding-masked softmax(q k^T / sqrt(dh)) @ v ;
  out = heads @ W_out.T + b_out

Sharding: tensor-parallel over heads. 16 heads / 8 cores = 2 heads per core.
Each core computes its 2 heads' QKV projection, attention, and a partial
output projection (y_partial = O_heads @ W_out[:, head_cols].T); the host
sums the 8 partials and adds b_out.

Device-side layout choices:
  - QKV matmul in "family B" layout: out[tokens(128p), 384 features] so the
    rotary pair-shuffle is a cheap strided access along the free axis.
  - q,k transposed on the PE to "family A" [dh, tokens] for the attention
    matmuls (contraction over dh must sit on partitions).
  - Attention computed transposed: S^T[k, q] = K Q^T, so the probability
    tile P^T = exp(...) feeds the O matmul directly as the moving operand.
  - V gets an appended ones column, so O'^T = [V|1]^T P^T yields both O^T
    and the softmax row-sums (Z) in one accumulation; normalization is a
    per-element multiply by broadcast 1/Z before the output projection.
  - Matmuls run as float32r (full-rate single-pass fp32 on the PE).
"""

import sys

import numpy as np

for _p in ("/opt/trn_rl_repo",):
    if _p not in sys.path:
        sys.path.insert(0, _p)

import concourse.bass as bass
import concourse.bacc as bacc
import concourse.mybir as mybir
import concourse.tile as tile
from concourse.bass_utils import run_bass_kernel_spmd
from concourse.masks import make_identity

# Problem shapes (hardcoded per contract).
B, N, D, H, DH = 2, 2048, 1024, 16, 64
NCORES = 8
HPC = H // NCORES            # heads per core
P = 128
NT = B * N                   # total tokens
SCALE = DH ** -0.5
FD = HPC * DH                # per-core features per tensor (128)
F3 = 3 * FD                  # 384
NEG = -1.0e30
NB = N // P                  # 16 token-blocks per batch
QC = N // 512                # 4 query chunks of 512 per batch

f32 = mybir.dt.float32
f32r = mybir.dt.float32r
AF = mybir.ActivationFunctionType
ALU = mybir.AluOpType


def _r(ap):
    """Bitcast an fp32 AP to float32r for full-rate PE matmuls."""
    return ap.bitcast(f32r)


def build_nc(use_f32r=True):
    nc = bacc.Bacc("TRN2", target_bir_lowering=False)
    fmm = f32r if use_f32r else f32
    mm = lambda ap: ap

    xT = nc.dram_tensor("xT", [D, NT], fmm, kind="ExternalInput")
    wqkvT = nc.dram_tensor("wqkvT", [D, F3], fmm, kind="ExternalInput")
    woT = nc.dram_tensor("woT", [FD, D], fmm, kind="ExternalInput")
    freq = nc.dram_tensor("freq", [N, DH], f32, kind="ExternalInput")
    madd = nc.dram_tensor("madd", [P, B * NB], f32, kind="ExternalInput")
    caus = nc.dram_tensor("caus", [P, P], f32, kind="ExternalInput")
    y = nc.dram_tensor("y", [NT, D], f32, kind="ExternalOutput")

    xT_r = xT.rearrange("(ko p) t -> p ko t", p=P)          # [128, 8, 4096]
    wq_r = wqkvT.rearrange("(ko p) f -> p ko f", p=P)       # [128, 8, 384]
    freq_r = freq.rearrange("(t p) d -> p t d", p=P)        # [128, 16, 64]

    with tile.TileContext(nc) as tc, \
            tc.tile_pool(name="const", bufs=1) as const, \
            tc.tile_pool(name="xp", bufs=2) as xp, \
            tc.tile_pool(name="qkp", bufs=1) as qkp, \
            tc.tile_pool(name="vfp", bufs=2) as vfp, \
            tc.tile_pool(name="qtp", bufs=1) as qtp, \
            tc.tile_pool(name="tmpp", bufs=3) as tmpp, \
            tc.tile_pool(name="ptp", bufs=4) as ptp, \
            tc.tile_pool(name="plp", bufs=2) as plp, \
            tc.tile_pool(name="smallp", bufs=3) as smallp, \
            tc.tile_pool(name="yp", bufs=4) as yp, \
            tc.tile_pool(name="psmm", bufs=4, space="PSUM") as psmm, \
            tc.tile_pool(name="pss", bufs=2, space="PSUM") as pss, \
            tc.tile_pool(name="pso", bufs=2, space="PSUM") as pso:

        # ---- constants / weights ----
        w_sb = const.tile([P, D // P, F3], fmm, tag="w")
        nc.sync.dma_start(w_sb[:, :, :], wq_r)
        wo_sb = const.tile([FD, D], fmm, tag="wo")
        nc.sync.dma_start(wo_sb[:, :], woT[:, :])
        caus_sb = const.tile([P, P], f32, tag="caus")
        nc.sync.dma_start(caus_sb[:, :], caus[:, :])
        madd_sb = const.tile([P, B * NB], f32, tag="madd")
        nc.sync.dma_start(madd_sb[:, :], madd[:, :])
        freq_sb = const.tile([P, NB, DH], f32, tag="freq")
        nc.sync.dma_start(freq_sb[:, :, :], freq_r)
        ident_f32 = const.tile([P, P], f32, tag="ident_f32")
        make_identity(nc, ident_f32)
        ident = const.tile([P, P], fmm, tag="ident")
        nc.vector.tensor_copy(ident, ident_f32)
        ones1 = const.tile([1, DH], f32, tag="ones1")
        nc.gpsimd.memset(ones1, 1.0)
        onecol = const.tile([P, 1], f32, tag="onecol")
        nc.gpsimd.memset(onecol, 1.0)

        # cos = sin(wrap(freq + pi/2)); sin_signed: negated at even dh positions.
        # Scalar-engine Sin needs inputs in [-pi, pi]; add_range_wrap handles
        # the shift + one-period wrap (valid for |freq| < 3*pi - shift).
        PI = float(np.pi)

        def range_wrap(out, xs, scratch):
            # out = xs - 2*pi * ((xs > pi) - (xs < -pi)) : one-period wrap
            g = scratch.tile([P, NB, DH], f32, tag="wrap_g")
            lo = scratch.tile([P, NB, DH], f32, tag="wrap_l")
            del scratch
            nc.vector.tensor_scalar(g, xs, PI, None, ALU.is_gt)
            nc.vector.tensor_scalar(lo, xs, -PI, None, ALU.is_lt)
            nc.vector.tensor_tensor(g, g, lo, ALU.subtract)
            nc.vector.scalar_tensor_tensor(out, g, -2 * PI, xs, ALU.mult, ALU.add)

        wrap_s = const.tile([P, NB, DH], f32, tag="wrap_s")
        range_wrap(wrap_s, freq_sb, const)
        wrap_c = const.tile([P, NB, DH], f32, tag="wrap_c")
        shifted = const.tile([P, NB, DH], f32, tag="shifted")
        nc.vector.tensor_scalar(shifted, freq_sb, PI / 2, None, ALU.add)
        range_wrap(wrap_c, shifted, const)
        cos_sb = const.tile([P, NB, DH], f32, tag="cos")
        nc.scalar.activation(cos_sb, wrap_c, AF.Sin)
        sins_sb = const.tile([P, NB, DH], f32, tag="sins")
        nc.scalar.activation(sins_sb[:, :, 0::2], wrap_s[:, :, 0::2], AF.Sin,
                             scale=-1.0)
        nc.scalar.activation(sins_sb[:, :, 1::2], wrap_s[:, :, 1::2], AF.Sin,
                             scale=1.0)

        for b in range(B):
            # ================= Phase B: QKV projection + rotary ============
            qkB = qkp.tile([P, NB, 2 * FD], fmm, tag="qkB")     # q01|k01
            vfB = vfp.tile([P, NB, HPC * (DH + 1)], fmm, tag="vfB")
            # ones column at position 64 of each head's 65-wide group
            # (copy-cast from an f32 const: memset can't encode float32r)
            nc.vector.tensor_copy(vfB[:, :, DH::DH + 1],
                                  onecol[:, None, :].to_broadcast([P, NB, HPC]))

            for c in range(N // 512):                # 512-token chunks
                x_sb = xp.tile([P, D // P, 512], fmm, tag="x")
                tok0 = b * N + c * 512
                nc.sync.dma_start(x_sb[:, :, :], xT_r[:, :, tok0:tok0 + 512])
                for tb in range(4):
                    t = c * 4 + tb                   # token-block in batch
                    qkv_ps = psmm.tile([P, F3], f32, tag="mm")
                    for ko in range(D // P):
                        nc.tensor.matmul(
                            qkv_ps,
                            mm(x_sb[:, ko, tb * P:(tb + 1) * P]),
                            mm(w_sb[:, ko, :]),
                            start=(ko == 0), stop=(ko == D // P - 1),
                        )
                    ps_g = qkv_ps.rearrange("p (g d) -> p g d", g=6)
                    # pass 1: tmp = pairswap(qkv) * sin_signed
                    tmp = tmpp.tile([P, F3], f32, tag="tmp")
                    tmp_g = tmp.rearrange("p (g d) -> p g d", g=6)
                    se = sins_sb[:, t, 0::2][:, None, :].to_broadcast([P, 6, DH // 2])
                    so = sins_sb[:, t, 1::2][:, None, :].to_broadcast([P, 6, DH // 2])
                    nc.vector.tensor_tensor(tmp_g[:, :, 0::2], ps_g[:, :, 1::2], se, ALU.mult)
                    nc.vector.tensor_tensor(tmp_g[:, :, 1::2], ps_g[:, :, 0::2], so, ALU.mult)
                    # pass 2: cosq = qkv * cos
                    cosq = tmpp.tile([P, F3], f32, tag="cosq")
                    cosq_g = cosq.rearrange("p (g d) -> p g d", g=6)
                    cb = cos_sb[:, t, :][:, None, :].to_broadcast([P, 6, DH])
                    nc.vector.tensor_tensor(cosq_g, ps_g, cb, ALU.mult)
                    # pass 3: rotated = tmp + cosq (q,k -> qkB; v -> vfB)
                    nc.gpsimd.tensor_tensor(qkB[:, t, :], tmp[:, 0:2 * FD],
                                            cosq[:, 0:2 * FD], ALU.add)
                    vf_v = vfB[:, t, :].rearrange("p (h c) -> p h c", h=HPC)[:, :, 0:DH]
                    tmp_v = tmp[:, 2 * FD:F3].rearrange("p (h d) -> p h d", h=HPC)
                    cos_v = cosq[:, 2 * FD:F3].rearrange("p (h d) -> p h d", h=HPC)
                    nc.gpsimd.tensor_tensor(vf_v, tmp_v, cos_v, ALU.add)

            # ================= Phase C: transpose q,k to [dh, tok] =========
            QT = qtp.tile([P, N], fmm, tag="QT")
            KT = qtp.tile([P, N], fmm, tag="KT")
            for t in range(NB):
                for which, dst in ((0, QT), (1, KT)):
                    tr_ps = psmm.tile([P, P], fmm, tag="mm")
                    nc.tensor.transpose(
                        tr_ps, qkB[:, t, which * FD:(which + 1) * FD], ident)
                    nc.scalar.copy(dst[:, t * P:(t + 1) * P], tr_ps)

            # ================= Phase D: attention ==========================
            PL = plp.tile([P, N], fmm, tag="PL")    # normalized O^T, 2 heads
            for h in range(HPC):
                Qh = QT[h * DH:(h + 1) * DH, :]
                Kh = KT[h * DH:(h + 1) * DH, :]
                for qc in range(QC):
                    O_ps = pso.tile([DH + 1, 512], f32, tag="o")
                    for kb in range(4 * qc + 4):
                        qs = max(kb * P, 512 * qc)
                        off = qs - 512 * qc
                        w = 512 - off
                        S_t = pss.tile([P, 512], f32, tag="s")
                        nc.tensor.matmul(S_t[:, :w], mm(Kh[:, kb * P:(kb + 1) * P]),
                                         mm(Qh[:, qs:qs + w]), start=True, stop=True)
                        if kb >= 4 * qc:  # chunk starts at the diagonal block
                            nc.vector.tensor_tensor(S_t[:, 0:P], S_t[:, 0:P],
                                                    caus_sb, ALU.add)
                        pt = ptp.tile([P, 512], fmm, tag="pt")
                        col = b * NB + kb
                        nc.scalar.activation(pt[:, :w], S_t[:, :w], AF.Exp,
                                             bias=madd_sb[:, col:col + 1],
                                             scale=SCALE)
                        nc.tensor.matmul(
                            O_ps[:, off:512],
                            mm(vfB[:, kb, h * (DH + 1):(h + 1) * (DH + 1)]),
                            mm(pt[:, :w]),
                            start=(kb == 0), stop=(kb == 4 * qc + 3),
                        )
                    # normalize: PL[h] = O^T * broadcast(1/Z)
                    rz = smallp.tile([1, 512], f32, tag="rz")
                    nc.vector.reciprocal(rz, O_ps[DH:DH + 1, :])
                    bc_ps = psmm.tile([DH, 512], f32, tag="mm")
                    nc.tensor.matmul(bc_ps, ones1, rz, start=True, stop=True)
                    rb = smallp.tile([DH, 512], f32, tag="rb")
                    nc.scalar.copy(rb, bc_ps)
                    nc.vector.tensor_tensor(
                        PL[h * DH:(h + 1) * DH, 512 * qc:512 * (qc + 1)],
                        O_ps[0:DH, :], rb, ALU.mult)

            # ================= Phase E: output projection ==================
            for t in range(NB):
                for dc in range(2):
                    y_ps = psmm.tile([P, 512], f32, tag="mm")
                    nc.tensor.matmul(y_ps, mm(PL[:, t * P:(t + 1) * P]),
                                     mm(wo_sb[:, dc * 512:(dc + 1) * 512]),
                                     start=True, stop=True)
                    y_sb = yp.tile([P, 512], f32, tag="ysb")
                    if dc == 0:
                        nc.vector.tensor_copy(y_sb, y_ps)
                    else:
                        nc.scalar.copy(y_sb, y_ps)
                    r0 = b * N + t * P
                    nc.sync.dma_start(y[r0:r0 + P, dc * 512:(dc + 1) * 512], y_sb)

    nc.compile()
    return nc


def prep_inputs(x, mask, rotary_pos_emb, W_qkv, W_out):
    """Host-side shard prep: per-core input dicts (layout only + mask encode)."""
    x = np.asarray(x, dtype=np.float32)
    W_qkv = np.asarray(W_qkv, dtype=np.float32)
    W_out = np.asarray(W_out, dtype=np.float32)
    rope = np.asarray(rotary_pos_emb, dtype=np.float32)
    mask = np.asarray(mask)

    xT = np.ascontiguousarray(x.reshape(NT, D).T)
    madd = np.where(mask, np.float32(0.0), np.float32(NEG)).astype(np.float32)
    madd_dev = np.ascontiguousarray(
        madd.reshape(B, NB, P).transpose(2, 0, 1).reshape(P, B * NB))
    kidx = np.arange(P)[:, None]
    qidx = np.arange(P)[None, :]
    caus = np.where(qidx >= kidx, np.float32(0.0), np.float32(NEG)).astype(np.float32)
    freq = np.ascontiguousarray(rope[-N:, :])

    in_maps = []
    for c in range(NCORES):
        rows = []
        for tsel in range(3):                      # q, k, v row blocks
            for h in (HPC * c, HPC * c + 1):
                o = tsel * H * DH + h * DH
                rows.append(W_qkv[o:o + DH, :])
        wqkvT = np.ascontiguousarray(np.concatenate(rows, axis=0).T)
        woT = np.ascontiguousarray(W_out[:, FD * c:FD * (c + 1)].T)
        in_maps.append({
            "xT": xT, "wqkvT": wqkvT, "woT": woT,
            "freq": freq, "madd": madd_dev, "caus": caus,
        })
    return in_maps


def _ensure_ntff_hook():
    """Install antenv.axon_hooks + the ctypes NTFF profile hook if the image
    lacks them (needed only for trace=True timing runs, not for kernel())."""
    import types
    try:
        from antenv.axon_hooks import get_axon_ntff_profile_hook  # noqa: F401
        return
    except ImportError:
        pass
    try:
        import antenv
        mod = types.ModuleType("antenv.axon_hooks")
        _state = {"hook": None}

        def set_axon_ntff_profile_hook(h):
            _state["hook"] = h

        def get_axon_ntff_profile_hook():
            return _state["hook"]

        mod.set_axon_ntff_profile_hook = set_axon_ntff_profile_hook
        mod.get_axon_ntff_profile_hook = get_axon_ntff_profile_hook
        sys.modules["antenv.axon_hooks"] = mod
        antenv.axon_hooks = mod
        from trn_agent_boot.trn_boot import _ntff_profile_via_ctypes
        hook = _ntff_profile_via_ctypes("/opt/axon/libaxon_pjrt.so")
        if hook is not None:
            set_axon_ntff_profile_hook(hook)
    except Exception as e:  # degrade to untimed runs
        print(f"ntff hook install failed: {e!r}", file=sys.stderr)


_NC_CACHE = {}


def _get_nc(use_f32r=True):
    key = bool(use_f32r)
    if key not in _NC_CACHE:
        _NC_CACHE[key] = build_nc(use_f32r=key)
    return _NC_CACHE[key]


def run_cores(in_maps, trace=False, use_f32r=True):
    if trace:
        _ensure_ntff_hook()
    nc = _get_nc(use_f32r)
    res = run_bass_kernel_spmd(
        nc, in_maps, core_ids=list(range(NCORES)), trace=trace,
        trace_cores=list(range(NCORES)) if trace else None,
    )
    return res


def kernel(x, mask, rotary_pos_emb, W_qkv, W_out, b_out):
    in_maps = prep_inputs(x, mask, rotary_pos_emb, W_qkv, W_out)
    res = run_cores(in_maps, trace=False)
    y = np.zeros((NT, D), dtype=np.float32)
    for r in res.results:
        y += r["y"]
    y += np.asarray(b_out, dtype=np.float32)[None, :]
    return y.reshape(B, N, D)


# revision 15
# speedup vs baseline: 1.2100x; 1.2100x over previous
"""Trainium2 Bass kernel for nn_Attention_25383256719981.

Dense transformer attention block:
  qkv = x @ W_qkv.T ; rotary(q,k,v) ; causal+padding-masked softmax(q k^T / sqrt(dh)) @ v ;
  out = heads @ W_out.T + b_out

Sharding: tensor-parallel over heads. 16 heads / 8 cores = 2 heads per core.
Each core computes its 2 heads' QKV projection, attention, and a partial
output projection (y_partial = O_heads @ W_out[:, head_cols].T); the host
sums the 8 partials and adds b_out.

Device-side layout choices:
  - QKV matmul in "family B" layout: out[tokens(128p), 384 features] so the
    rotary pair-shuffle is a cheap strided access along the free axis.
  - q,k transposed on the PE to "family A" [dh, tokens] for the attention
    matmuls (contraction over dh must sit on partitions).
  - Attention computed transposed: S^T[k, q] = K Q^T, so the probability
    tile P^T = exp(...) feeds the O matmul directly as the moving operand.
  - V gets an appended ones column, so O'^T = [V|1]^T P^T yields both O^T
    and the softmax row-sums (Z) in one accumulation; normalization is a
    per-element multiply by broadcast 1/Z before the output projection.
  - Matmuls run as float32r (full-rate single-pass fp32 on the PE).
"""

import sys

import numpy as np

for _p in ("/opt/trn_rl_repo",):
    if _p not in sys.path:
        sys.path.insert(0, _p)

import concourse.bass as bass
import concourse.bacc as bacc
import concourse.mybir as mybir
import concourse.tile as tile
from concourse.bass_utils import run_bass_kernel_spmd
from concourse.masks import make_identity

# Problem shapes (hardcoded per contract).
B, N, D, H, DH = 2, 2048, 1024, 16, 64
NCORES = 8
HPC = H // NCORES            # heads per core
P = 128
NT = B * N                   # total tokens
SCALE = DH ** -0.5
FD = HPC * DH                # per-core features per tensor (128)
F3 = 3 * FD                  # 384
NEG = -1.0e30
NB = N // P                  # 16 token-blocks per batch
QC = N // 512                # 4 query chunks of 512 per batch

f32 = mybir.dt.float32
f32r = mybir.dt.float32r
f16 = mybir.dt.float16
AF = mybir.ActivationFunctionType
ALU = mybir.AluOpType


def _r(ap):
    """Bitcast an fp32 AP to float32r for full-rate PE matmuls."""
    return ap.bitcast(f32r)


def build_nc(dt_mode="f16"):
    nc = bacc.Bacc("TRN2", target_bir_lowering=False)
    fmm = {"f16": f16, "f32r": f32r, "f32": f32}[dt_mode]
    mm = lambda ap: ap

    xT = nc.dram_tensor("xT", [D, NT], fmm, kind="ExternalInput")
    wqkvT = nc.dram_tensor("wqkvT", [D, F3], fmm, kind="ExternalInput")
    woT = nc.dram_tensor("woT", [FD, D], fmm, kind="ExternalInput")
    freq = nc.dram_tensor("freq", [N, DH], f32, kind="ExternalInput")
    madd = nc.dram_tensor("madd", [P, B * NB], f32, kind="ExternalInput")
    caus = nc.dram_tensor("caus", [P, P], f32, kind="ExternalInput")
    y = nc.dram_tensor("y", [NT, D], f32, kind="ExternalOutput")

    xT_r = xT.rearrange("(ko p) t -> p ko t", p=P)          # [128, 8, 4096]
    wq_r = wqkvT.rearrange("(ko p) f -> p ko f", p=P)       # [128, 8, 384]
    freq_r = freq.rearrange("(t p) d -> p t d", p=P)        # [128, 16, 64]

    with tile.TileContext(nc) as tc, \
            tc.tile_pool(name="const", bufs=1) as const, \
            tc.tile_pool(name="xp", bufs=2) as xp, \
            tc.tile_pool(name="qkp", bufs=1) as qkp, \
            tc.tile_pool(name="vfp", bufs=2) as vfp, \
            tc.tile_pool(name="qtp", bufs=1) as qtp, \
            tc.tile_pool(name="tmpp", bufs=3) as tmpp, \
            tc.tile_pool(name="ptp", bufs=4) as ptp, \
            tc.tile_pool(name="plp", bufs=2) as plp, \
            tc.tile_pool(name="smallp", bufs=3) as smallp, \
            tc.tile_pool(name="yp", bufs=4) as yp, \
            tc.tile_pool(name="psmm", bufs=4, space="PSUM") as psmm, \
            tc.tile_pool(name="pss", bufs=2, space="PSUM") as pss, \
            tc.tile_pool(name="pso", bufs=2, space="PSUM") as pso:

        # ---- constants / weights ----
        w_sb = const.tile([P, D // P, F3], fmm, tag="w")
        nc.sync.dma_start(w_sb[:, :, :], wq_r)
        wo_sb = const.tile([FD, D], fmm, tag="wo")
        nc.sync.dma_start(wo_sb[:, :], woT[:, :])
        caus_sb = const.tile([P, P], f32, tag="caus")
        nc.sync.dma_start(caus_sb[:, :], caus[:, :])
        madd_sb = const.tile([P, B * NB], f32, tag="madd")
        nc.sync.dma_start(madd_sb[:, :], madd[:, :])
        freq_sb = const.tile([P, NB, DH], f32, tag="freq")
        nc.sync.dma_start(freq_sb[:, :, :], freq_r)
        if mybir.dt.size(fmm) != 2:
            ident_f32 = const.tile([P, P], f32, tag="ident_f32")
            make_identity(nc, ident_f32)
            ident = const.tile([P, P], fmm, tag="ident")
            nc.vector.tensor_copy(ident, ident_f32)
        onecol = const.tile([P, 1], f32, tag="onecol")
        nc.gpsimd.memset(onecol, 1.0)

        # cos = sin(wrap(freq + pi/2)); sin_signed: negated at even dh positions.
        # Scalar-engine Sin needs inputs in [-pi, pi]; add_range_wrap handles
        # the shift + one-period wrap (valid for |freq| < 3*pi - shift).
        PI = float(np.pi)

        def range_wrap(out, xs, scratch):
            # out = xs - 2*pi * ((xs > pi) - (xs < -pi)) : one-period wrap
            g = scratch.tile([P, NB, DH], f32, tag="wrap_g")
            lo = scratch.tile([P, NB, DH], f32, tag="wrap_l")
            del scratch
            nc.vector.tensor_scalar(g, xs, PI, None, ALU.is_gt)
            nc.vector.tensor_scalar(lo, xs, -PI, None, ALU.is_lt)
            nc.vector.tensor_tensor(g, g, lo, ALU.subtract)
            nc.vector.scalar_tensor_tensor(out, g, -2 * PI, xs, ALU.mult, ALU.add)

        wrap_s = const.tile([P, NB, DH], f32, tag="wrap_s")
        range_wrap(wrap_s, freq_sb, const)
        wrap_c = const.tile([P, NB, DH], f32, tag="wrap_c")
        shifted = const.tile([P, NB, DH], f32, tag="shifted")
        nc.vector.tensor_scalar(shifted, freq_sb, PI / 2, None, ALU.add)
        range_wrap(wrap_c, shifted, const)
        cos_sb = const.tile([P, NB, DH], f32, tag="cos")
        nc.scalar.activation(cos_sb, wrap_c, AF.Sin)
        sins_sb = const.tile([P, NB, DH], f32, tag="sins")
        nc.scalar.activation(sins_sb[:, :, 0::2], wrap_s[:, :, 0::2], AF.Sin,
                             scale=-1.0)
        nc.scalar.activation(sins_sb[:, :, 1::2], wrap_s[:, :, 1::2], AF.Sin,
                             scale=1.0)

        for b in range(B):
            # ================= Phase B: QKV projection + rotary ============
            qkB = qkp.tile([P, NB, 2 * FD], fmm, tag="qkB")     # q01|k01
            vfB = vfp.tile([P, NB, HPC * (DH + 1)], fmm, tag="vfB")
            # ones column at position 64 of each head's 65-wide group
            # (copy-cast from an f32 const: memset can't encode float32r)
            nc.vector.tensor_copy(vfB[:, :, DH::DH + 1],
                                  onecol[:, None, :].to_broadcast([P, NB, HPC]))

            for c in range(N // 512):                # 512-token chunks
                x_sb = xp.tile([P, D // P, 512], fmm, tag="x")
                tok0 = b * N + c * 512
                nc.sync.dma_start(x_sb[:, :, :], xT_r[:, :, tok0:tok0 + 512])
                for tb in range(4):
                    t = c * 4 + tb                   # token-block in batch
                    qkv_ps = psmm.tile([P, F3], f32, tag="mm")
                    for ko in range(D // P):
                        nc.tensor.matmul(
                            qkv_ps,
                            mm(x_sb[:, ko, tb * P:(tb + 1) * P]),
                            mm(w_sb[:, ko, :]),
                            start=(ko == 0), stop=(ko == D // P - 1),
                        )
                    ps_g = qkv_ps.rearrange("p (g d) -> p g d", g=6)
                    # pass 1: tmp = pairswap(qkv) * sin_signed
                    tmp = tmpp.tile([P, F3], f32, tag="tmp")
                    tmp_g = tmp.rearrange("p (g d) -> p g d", g=6)
                    se = sins_sb[:, t, 0::2][:, None, :].to_broadcast([P, 6, DH // 2])
                    so = sins_sb[:, t, 1::2][:, None, :].to_broadcast([P, 6, DH // 2])
                    nc.vector.tensor_tensor(tmp_g[:, :, 0::2], ps_g[:, :, 1::2], se, ALU.mult)
                    nc.vector.tensor_tensor(tmp_g[:, :, 1::2], ps_g[:, :, 0::2], so, ALU.mult)
                    # pass 2: cosq = qkv * cos
                    cosq = tmpp.tile([P, F3], f32, tag="cosq")
                    cosq_g = cosq.rearrange("p (g d) -> p g d", g=6)
                    cb = cos_sb[:, t, :][:, None, :].to_broadcast([P, 6, DH])
                    nc.vector.tensor_tensor(cosq_g, ps_g, cb, ALU.mult)
                    # pass 3: rotated = tmp + cosq (q,k -> qkB; v -> vfB)
                    nc.gpsimd.tensor_tensor(qkB[:, t, :], tmp[:, 0:2 * FD],
                                            cosq[:, 0:2 * FD], ALU.add)
                    vf_v = vfB[:, t, :].rearrange("p (h c) -> p h c", h=HPC)[:, :, 0:DH]
                    tmp_v = tmp[:, 2 * FD:F3].rearrange("p (h d) -> p h d", h=HPC)
                    cos_v = cosq[:, 2 * FD:F3].rearrange("p (h d) -> p h d", h=HPC)
                    nc.gpsimd.tensor_tensor(vf_v, tmp_v, cos_v, ALU.add)

            # ================= Phase C: transpose q,k to [dh, tok] =========
            QT = qtp.tile([P, N], fmm, tag="QT")
            KT = qtp.tile([P, N], fmm, tag="KT")
            for t in range(NB):
                for which, dst in ((0, QT), (1, KT)):
                    src = qkB[:, t, which * FD:(which + 1) * FD]
                    if mybir.dt.size(fmm) == 2:
                        nc.sync.dma_start_transpose(
                            dst[:, t * P:(t + 1) * P], src)
                    else:
                        tr_ps = psmm.tile([P, P], fmm, tag="mm")
                        nc.tensor.transpose(tr_ps, src, ident)
                        nc.scalar.copy(dst[:, t * P:(t + 1) * P], tr_ps)

            # ================= Phase D: attention ==========================
            PL = plp.tile([P, N], fmm, tag="PL")    # normalized O^T, 2 heads
            for h in range(HPC):
                Qh = QT[h * DH:(h + 1) * DH, :]
                Kh = KT[h * DH:(h + 1) * DH, :]
                for qc in range(QC):
                    O_ps = pso.tile([DH + 1, 512], f32, tag="o")
                    for kb in range(4 * qc + 4):
                        qs = max(kb * P, 512 * qc)
                        off = qs - 512 * qc
                        w = 512 - off
                        S_t = pss.tile([P, 512], f32, tag="s")
                        nc.tensor.matmul(S_t[:, :w], mm(Kh[:, kb * P:(kb + 1) * P]),
                                         mm(Qh[:, qs:qs + w]), start=True, stop=True)
                        if kb >= 4 * qc:  # chunk starts at the diagonal block
                            nc.vector.tensor_tensor(S_t[:, 0:P], S_t[:, 0:P],
                                                    caus_sb, ALU.add)
                        pt = ptp.tile([P, 512], fmm, tag="pt")
                        col = b * NB + kb
                        nc.scalar.activation(pt[:, :w], S_t[:, :w], AF.Exp,
                                             bias=madd_sb[:, col:col + 1],
                                             scale=SCALE)
                        nc.tensor.matmul(
                            O_ps[:, off:512],
                            mm(vfB[:, kb, h * (DH + 1):(h + 1) * (DH + 1)]),
                            mm(pt[:, :w]),
                            start=(kb == 0), stop=(kb == 4 * qc + 3),
                        )
                    # normalize: PL[h] = O^T * broadcast(1/Z)
                    rz = smallp.tile([1, 512], f32, tag="rz")
                    nc.vector.reciprocal(rz, O_ps[DH:DH + 1, :])
                    rb = smallp.tile([DH, 512], f32, tag="rb")
                    nc.gpsimd.partition_broadcast(rb, rz)
                    nc.vector.tensor_tensor(
                        PL[h * DH:(h + 1) * DH, 512 * qc:512 * (qc + 1)],
                        O_ps[0:DH, :], rb, ALU.mult)

            # ================= Phase E: output projection ==================
            for t in range(NB):
                for dc in range(2):
                    y_ps = psmm.tile([P, 512], f32, tag="mm")
                    nc.tensor.matmul(y_ps, mm(PL[:, t * P:(t + 1) * P]),
                                     mm(wo_sb[:, dc * 512:(dc + 1) * 512]),
                                     start=True, stop=True)
                    y_sb = yp.tile([P, 512], f32, tag="ysb")
                    if dc == 0:
                        nc.vector.tensor_copy(y_sb, y_ps)
                    else:
                        nc.scalar.copy(y_sb, y_ps)
                    r0 = b * N + t * P
                    nc.sync.dma_start(y[r0:r0 + P, dc * 512:(dc + 1) * 512], y_sb)

    nc.compile()
    return nc


def prep_inputs(x, mask, rotary_pos_emb, W_qkv, W_out, dt_mode="f16"):
    """Host-side shard prep: per-core input dicts (layout only + mask encode)."""
    np_mm = np.float16 if dt_mode == "f16" else np.float32
    x = np.asarray(x, dtype=np.float32)
    W_qkv = np.asarray(W_qkv, dtype=np.float32)
    W_out = np.asarray(W_out, dtype=np.float32)
    rope = np.asarray(rotary_pos_emb, dtype=np.float32)
    mask = np.asarray(mask)

    xT = np.ascontiguousarray(x.reshape(NT, D).T.astype(np_mm))
    madd = np.where(mask, np.float32(0.0), np.float32(NEG)).astype(np.float32)
    madd_dev = np.ascontiguousarray(
        madd.reshape(B, NB, P).transpose(2, 0, 1).reshape(P, B * NB))
    kidx = np.arange(P)[:, None]
    qidx = np.arange(P)[None, :]
    caus = np.where(qidx >= kidx, np.float32(0.0), np.float32(NEG)).astype(np.float32)
    freq = np.ascontiguousarray(rope[-N:, :])

    in_maps = []
    for c in range(NCORES):
        rows = []
        for tsel in range(3):                      # q, k, v row blocks
            for h in (HPC * c, HPC * c + 1):
                o = tsel * H * DH + h * DH
                rows.append(W_qkv[o:o + DH, :])
        wqkvT = np.ascontiguousarray(np.concatenate(rows, axis=0).T.astype(np_mm))
        woT = np.ascontiguousarray(W_out[:, FD * c:FD * (c + 1)].T.astype(np_mm))
        in_maps.append({
            "xT": xT, "wqkvT": wqkvT, "woT": woT,
            "freq": freq, "madd": madd_dev, "caus": caus,
        })
    return in_maps


def _ensure_ntff_hook():
    """Install antenv.axon_hooks + the ctypes NTFF profile hook if the image
    lacks them (needed only for trace=True timing runs, not for kernel())."""
    import types
    try:
        from antenv.axon_hooks import get_axon_ntff_profile_hook  # noqa: F401
        return
    except ImportError:
        pass
    try:
        import antenv
        mod = types.ModuleType("antenv.axon_hooks")
        _state = {"hook": None}

        def set_axon_ntff_profile_hook(h):
            _state["hook"] = h

        def get_axon_ntff_profile_hook():
            return _state["hook"]

        mod.set_axon_ntff_profile_hook = set_axon_ntff_profile_hook
        mod.get_axon_ntff_profile_hook = get_axon_ntff_profile_hook
        sys.modules["antenv.axon_hooks"] = mod
        antenv.axon_hooks = mod
        from trn_agent_boot.trn_boot import _ntff_profile_via_ctypes
        hook = _ntff_profile_via_ctypes("/opt/axon/libaxon_pjrt.so")
        if hook is not None:
            set_axon_ntff_profile_hook(hook)
    except Exception as e:  # degrade to untimed runs
        print(f"ntff hook install failed: {e!r}", file=sys.stderr)


_NC_CACHE = {}


def _get_nc(dt_mode="f16"):
    if dt_mode not in _NC_CACHE:
        _NC_CACHE[dt_mode] = build_nc(dt_mode=dt_mode)
    return _NC_CACHE[dt_mode]


def run_cores(in_maps, trace=False, dt_mode="f16"):
    if trace:
        _ensure_ntff_hook()
    nc = _get_nc(dt_mode)
    res = run_bass_kernel_spmd(
        nc, in_maps, core_ids=list(range(NCORES)), trace=trace,
        trace_cores=list(range(NCORES)) if trace else None,
    )
    return res


DT_MODE = "f16"


def kernel(x, mask, rotary_pos_emb, W_qkv, W_out, b_out):
    in_maps = prep_inputs(x, mask, rotary_pos_emb, W_qkv, W_out, dt_mode=DT_MODE)
    res = run_cores(in_maps, trace=False, dt_mode=DT_MODE)
    y = np.zeros((NT, D), dtype=np.float32)
    for r in res.results:
        y += r["y"]
    y += np.asarray(b_out, dtype=np.float32)[None, :]
    return y.reshape(B, N, D)


# revision 17
# speedup vs baseline: 1.3707x; 1.1328x over previous
"""Trainium2 Bass kernel for nn_Attention_25383256719981.

Dense transformer attention block:
  qkv = x @ W_qkv.T ; rotary(q,k,v) ; causal+padding-masked softmax(q k^T / sqrt(dh)) @ v ;
  out = heads @ W_out.T + b_out

Sharding: tensor-parallel over heads. 16 heads / 8 cores = 2 heads per core.
Each core computes its 2 heads' QKV projection, attention, and a partial
output projection (y_partial = O_heads @ W_out[:, head_cols].T); the host
sums the 8 partials and adds b_out.

Device-side layout choices:
  - QKV matmul in "family B" layout: out[tokens(128p), 384 features] so the
    rotary pair-shuffle is a cheap strided access along the free axis.
  - q,k transposed on the PE to "family A" [dh, tokens] for the attention
    matmuls (contraction over dh must sit on partitions).
  - Attention computed transposed: S^T[k, q] = K Q^T, so the probability
    tile P^T = exp(...) feeds the O matmul directly as the moving operand.
  - V gets an appended ones column, so O'^T = [V|1]^T P^T yields both O^T
    and the softmax row-sums (Z) in one accumulation; normalization is a
    per-element multiply by broadcast 1/Z before the output projection.
  - Matmuls run as float32r (full-rate single-pass fp32 on the PE).
"""

import sys

import numpy as np

for _p in ("/opt/trn_rl_repo",):
    if _p not in sys.path:
        sys.path.insert(0, _p)

import concourse.bass as bass
import concourse.bacc as bacc
import concourse.mybir as mybir
import concourse.tile as tile
from concourse.bass_utils import run_bass_kernel_spmd
from concourse.masks import make_identity

# Problem shapes (hardcoded per contract).
B, N, D, H, DH = 2, 2048, 1024, 16, 64
NCORES = 8
HPC = H // NCORES            # heads per core
P = 128
NT = B * N                   # total tokens
SCALE = DH ** -0.5
FD = HPC * DH                # per-core features per tensor (128)
F3 = 3 * FD                  # 384
NEG = -1.0e30
NB = N // P                  # 16 token-blocks per batch
QC = N // 512                # 4 query chunks of 512 per batch

f32 = mybir.dt.float32
f32r = mybir.dt.float32r
f16 = mybir.dt.float16
AF = mybir.ActivationFunctionType
ALU = mybir.AluOpType


def _r(ap):
    """Bitcast an fp32 AP to float32r for full-rate PE matmuls."""
    return ap.bitcast(f32r)


def build_nc(dt_mode="f16"):
    nc = bacc.Bacc("TRN2", target_bir_lowering=False)
    fmm = {"f16": f16, "f32r": f32r, "f32": f32}[dt_mode]
    mm = lambda ap: ap

    xT = nc.dram_tensor("xT", [D, NT], fmm, kind="ExternalInput")
    wqkvT = nc.dram_tensor("wqkvT", [D, F3], fmm, kind="ExternalInput")
    woT = nc.dram_tensor("woT", [FD, D], fmm, kind="ExternalInput")
    freq = nc.dram_tensor("freq", [N, DH], f32, kind="ExternalInput")
    madd = nc.dram_tensor("madd", [P, B * NB], f32, kind="ExternalInput")
    caus = nc.dram_tensor("caus", [P, P], f32, kind="ExternalInput")
    y = nc.dram_tensor("y", [NT, D], f32, kind="ExternalOutput")

    xT_r = xT.rearrange("(ko p) t -> p ko t", p=P)          # [128, 8, 4096]
    wq_r = wqkvT.rearrange("(ko p) f -> p ko f", p=P)       # [128, 8, 384]
    freq_r = freq.rearrange("(t p) d -> p t d", p=P)        # [128, 16, 64]

    with tile.TileContext(nc) as tc, \
            tc.tile_pool(name="const", bufs=1) as const, \
            tc.tile_pool(name="xp", bufs=2) as xp, \
            tc.tile_pool(name="qkp", bufs=1) as qkp, \
            tc.tile_pool(name="vfp", bufs=2) as vfp, \
            tc.tile_pool(name="qtp", bufs=1) as qtp, \
            tc.tile_pool(name="tmpp", bufs=3) as tmpp, \
            tc.tile_pool(name="ptp", bufs=4) as ptp, \
            tc.tile_pool(name="plp", bufs=2) as plp, \
            tc.tile_pool(name="smallp", bufs=3) as smallp, \
            tc.tile_pool(name="yp", bufs=4) as yp, \
            tc.tile_pool(name="psmm", bufs=2, space="PSUM") as psmm, \
            tc.tile_pool(name="pss", bufs=2, space="PSUM") as pss, \
            tc.tile_pool(name="pstr", bufs=2, space="PSUM") as pstr, \
            tc.tile_pool(name="pso", bufs=2, space="PSUM") as pso:

        # ---- constants / weights ----
        w_sb = const.tile([P, D // P, F3], fmm, tag="w")
        nc.sync.dma_start(w_sb[:, :, :], wq_r)
        wo_sb = const.tile([FD, D], fmm, tag="wo")
        nc.sync.dma_start(wo_sb[:, :], woT[:, :])
        caus_sb = const.tile([P, P], f32, tag="caus")
        nc.sync.dma_start(caus_sb[:, :], caus[:, :])
        madd_sb = const.tile([P, B * NB], f32, tag="madd")
        nc.sync.dma_start(madd_sb[:, :], madd[:, :])
        freq_sb = const.tile([P, NB, DH], f32, tag="freq")
        nc.sync.dma_start(freq_sb[:, :, :], freq_r)
        ident_f32 = const.tile([P, P], f32, tag="ident_f32")
        make_identity(nc, ident_f32)
        ident = const.tile([P, P], fmm, tag="ident")
        nc.vector.tensor_copy(ident, ident_f32)
        onecol = const.tile([P, 1], f32, tag="onecol")
        nc.gpsimd.memset(onecol, 1.0)

        # cos = sin(wrap(freq + pi/2)); sin_signed: negated at even dh positions.
        # Scalar-engine Sin needs inputs in [-pi, pi]; add_range_wrap handles
        # the shift + one-period wrap (valid for |freq| < 3*pi - shift).
        PI = float(np.pi)

        def range_wrap(out, xs, scratch):
            # out = xs - 2*pi * ((xs > pi) - (xs < -pi)) : one-period wrap
            g = scratch.tile([P, NB, DH], f32, tag="wrap_g")
            lo = scratch.tile([P, NB, DH], f32, tag="wrap_l")
            del scratch
            nc.vector.tensor_scalar(g, xs, PI, None, ALU.is_gt)
            nc.vector.tensor_scalar(lo, xs, -PI, None, ALU.is_lt)
            nc.vector.tensor_tensor(g, g, lo, ALU.subtract)
            nc.vector.scalar_tensor_tensor(out, g, -2 * PI, xs, ALU.mult, ALU.add)

        wrap_s = const.tile([P, NB, DH], f32, tag="wrap_s")
        range_wrap(wrap_s, freq_sb, const)
        wrap_c = const.tile([P, NB, DH], f32, tag="wrap_c")
        shifted = const.tile([P, NB, DH], f32, tag="shifted")
        nc.vector.tensor_scalar(shifted, freq_sb, PI / 2, None, ALU.add)
        range_wrap(wrap_c, shifted, const)
        cos_sb = const.tile([P, NB, DH], f32, tag="cos")
        nc.scalar.activation(cos_sb, wrap_c, AF.Sin)
        sins_sb = const.tile([P, NB, DH], f32, tag="sins")
        nc.scalar.activation(sins_sb[:, :, 0::2], wrap_s[:, :, 0::2], AF.Sin,
                             scale=-1.0)
        nc.scalar.activation(sins_sb[:, :, 1::2], wrap_s[:, :, 1::2], AF.Sin,
                             scale=1.0)

        for b in range(B):
            # ================= Phase B: QKV projection + rotary ============
            qkB = qkp.tile([P, NB, 2 * FD], fmm, tag="qkB")     # q01|k01
            vfB = vfp.tile([P, NB, HPC * (DH + 1)], fmm, tag="vfB")
            # ones column at position 64 of each head's 65-wide group
            # (copy-cast from an f32 const: memset can't encode float32r)
            nc.vector.tensor_copy(vfB[:, :, DH::DH + 1],
                                  onecol[:, None, :].to_broadcast([P, NB, HPC]))

            for c in range(N // 512):                # 512-token chunks
                x_sb = xp.tile([P, D // P, 512], fmm, tag="x")
                tok0 = b * N + c * 512
                nc.sync.dma_start(x_sb[:, :, :], xT_r[:, :, tok0:tok0 + 512])
                for tb in range(4):
                    t = c * 4 + tb                   # token-block in batch
                    qkv_ps = psmm.tile([P, F3], f32, tag="mm")
                    for ko in range(D // P):
                        nc.tensor.matmul(
                            qkv_ps,
                            mm(x_sb[:, ko, tb * P:(tb + 1) * P]),
                            mm(w_sb[:, ko, :]),
                            start=(ko == 0), stop=(ko == D // P - 1),
                        )
                    ps_g = qkv_ps.rearrange("p (g d) -> p g d", g=6)
                    # pass 1: tmp = pairswap(qkv) * sin_signed
                    tmp = tmpp.tile([P, F3], f32, tag="tmp")
                    tmp_g = tmp.rearrange("p (g d) -> p g d", g=6)
                    se = sins_sb[:, t, 0::2][:, None, :].to_broadcast([P, 6, DH // 2])
                    so = sins_sb[:, t, 1::2][:, None, :].to_broadcast([P, 6, DH // 2])
                    nc.vector.tensor_tensor(tmp_g[:, :, 0::2], ps_g[:, :, 1::2], se, ALU.mult)
                    nc.vector.tensor_tensor(tmp_g[:, :, 1::2], ps_g[:, :, 0::2], so, ALU.mult)
                    # pass 2: cosq = qkv * cos
                    cosq = tmpp.tile([P, F3], f32, tag="cosq")
                    cosq_g = cosq.rearrange("p (g d) -> p g d", g=6)
                    cb = cos_sb[:, t, :][:, None, :].to_broadcast([P, 6, DH])
                    nc.vector.tensor_tensor(cosq_g, ps_g, cb, ALU.mult)
                    # pass 3: rotated = tmp + cosq (q,k -> qkB; v -> vfB)
                    nc.gpsimd.tensor_tensor(qkB[:, t, :], tmp[:, 0:2 * FD],
                                            cosq[:, 0:2 * FD], ALU.add)
                    vf_v = vfB[:, t, :].rearrange("p (h c) -> p h c", h=HPC)[:, :, 0:DH]
                    tmp_v = tmp[:, 2 * FD:F3].rearrange("p (h d) -> p h d", h=HPC)
                    cos_v = cosq[:, 2 * FD:F3].rearrange("p (h d) -> p h d", h=HPC)
                    nc.gpsimd.tensor_tensor(vf_v, tmp_v, cos_v, ALU.add)

            # ================= Phase C: transpose q,k to [dh, tok] =========
            QT = qtp.tile([P, N], fmm, tag="QT")
            KT = qtp.tile([P, N], fmm, tag="KT")
            for t in range(NB):
                for which, dst in ((0, QT), (1, KT)):
                    src = qkB[:, t, which * FD:(which + 1) * FD]
                    tr_ps = pstr.tile([P, P], fmm, tag="tr")
                    nc.tensor.transpose(tr_ps, src, ident)
                    if (t + which) % 2 == 0:
                        nc.vector.tensor_copy(dst[:, t * P:(t + 1) * P], tr_ps)
                    else:
                        nc.scalar.copy(dst[:, t * P:(t + 1) * P], tr_ps)

            # ================= Phase D: attention ==========================
            # Software-pipelined: the S matmul for chunk i+1 is emitted before
            # exp/O of chunk i so the PE never stalls on the ScalarE exp.
            PL = plp.tile([P, N], fmm, tag="PL")    # normalized O^T, 2 heads
            for h in range(HPC):
                Qh = QT[h * DH:(h + 1) * DH, :]
                Kh = KT[h * DH:(h + 1) * DH, :]
                O_tiles = {}

                def emit_S(qc, kb, h=h, Qh=Qh, Kh=Kh):
                    qs = max(kb * P, 512 * qc)
                    off = qs - 512 * qc
                    w = 512 - off
                    S_t = pss.tile([P, 512], f32, tag="s")
                    nc.tensor.matmul(S_t[:, :w], mm(Kh[:, kb * P:(kb + 1) * P]),
                                     mm(Qh[:, qs:qs + w]), start=True, stop=True)
                    if kb >= 4 * qc:  # chunk starts at the diagonal block
                        nc.vector.tensor_tensor(S_t[:, 0:P], S_t[:, 0:P],
                                                caus_sb, ALU.add)
                    return (qc, kb, off, w, S_t)

                def emit_expO(qc, kb, off, w, S_t, h=h, b=b):
                    pt = ptp.tile([P, 512], fmm, tag="pt")
                    col = b * NB + kb
                    nc.scalar.activation(pt[:, :w], S_t[:, :w], AF.Exp,
                                         bias=madd_sb[:, col:col + 1],
                                         scale=SCALE)
                    if kb == 0:
                        O_tiles[qc] = pso.tile([DH + 1, 512], f32, tag="o", name=f"O_{qc}")
                    O_ps = O_tiles[qc]
                    nc.tensor.matmul(
                        O_ps[:, off:512],
                        mm(vfB[:, kb, h * (DH + 1):(h + 1) * (DH + 1)]),
                        mm(pt[:, :w]),
                        start=(kb == 0), stop=(kb == 4 * qc + 3),
                    )
                    if kb == 4 * qc + 3:
                        # normalize: PL[h] = O^T * broadcast(1/Z).  1/Z via a
                        # fold to [128, 4] so the DVE reciprocal runs on all
                        # lanes instead of one.
                        zrow = smallp.tile([1, 512], f32, tag="zrow")
                        nc.scalar.copy(zrow, O_ps[DH:DH + 1, :])
                        zf = smallp.tile([P, 4], f32, tag="zf")
                        nc.sync.dma_start(
                            zf[:, :], zrow.rearrange("o (p c) -> o p c", p=P))
                        rf = smallp.tile([P, 4], f32, tag="rf")
                        nc.vector.reciprocal(rf, zf)
                        rrow = smallp.tile([1, 512], f32, tag="rrow")
                        nc.sync.dma_start(
                            rrow.rearrange("o (p c) -> o p c", p=P), rf[:, :])
                        rb = smallp.tile([DH, 512], f32, tag="rb")
                        nc.gpsimd.partition_broadcast(rb, rrow)
                        nc.vector.tensor_tensor(
                            PL[h * DH:(h + 1) * DH, 512 * qc:512 * (qc + 1)],
                            O_ps[0:DH, :], rb, ALU.mult)

                chunks = [(qc, kb) for qc in range(QC)
                          for kb in range(4 * qc + 4)]
                pend = None
                for qc, kb in chunks:
                    cur = emit_S(qc, kb)
                    if pend is not None:
                        emit_expO(*pend)
                    pend = cur
                emit_expO(*pend)

            # ================= Phase E: output projection ==================
            for t in range(NB):
                y_sb = yp.tile([P, D], f32, tag="ysb")
                for dc in range(2):
                    y_ps = psmm.tile([P, 512], f32, tag="mm")
                    nc.tensor.matmul(y_ps, mm(PL[:, t * P:(t + 1) * P]),
                                     mm(wo_sb[:, dc * 512:(dc + 1) * 512]),
                                     start=True, stop=True)
                    if dc == 0:
                        nc.vector.tensor_copy(y_sb[:, 0:512], y_ps)
                    else:
                        nc.scalar.copy(y_sb[:, 512:1024], y_ps)
                r0 = b * N + t * P
                nc.sync.dma_start(y[r0:r0 + P, :], y_sb)

    nc.compile()
    return nc


def prep_inputs(x, mask, rotary_pos_emb, W_qkv, W_out, dt_mode="f16"):
    """Host-side shard prep: per-core input dicts (layout only + mask encode)."""
    np_mm = np.float16 if dt_mode == "f16" else np.float32
    x = np.asarray(x, dtype=np.float32)
    W_qkv = np.asarray(W_qkv, dtype=np.float32)
    W_out = np.asarray(W_out, dtype=np.float32)
    rope = np.asarray(rotary_pos_emb, dtype=np.float32)
    mask = np.asarray(mask)

    xT = np.ascontiguousarray(x.reshape(NT, D).T.astype(np_mm))
    madd = np.where(mask, np.float32(0.0), np.float32(NEG)).astype(np.float32)
    madd_dev = np.ascontiguousarray(
        madd.reshape(B, NB, P).transpose(2, 0, 1).reshape(P, B * NB))
    kidx = np.arange(P)[:, None]
    qidx = np.arange(P)[None, :]
    caus = np.where(qidx >= kidx, np.float32(0.0), np.float32(NEG)).astype(np.float32)
    freq = np.ascontiguousarray(rope[-N:, :])

    in_maps = []
    for c in range(NCORES):
        rows = []
        for tsel in range(3):                      # q, k, v row blocks
            for h in (HPC * c, HPC * c + 1):
                o = tsel * H * DH + h * DH
                rows.append(W_qkv[o:o + DH, :])
        wqkvT = np.ascontiguousarray(np.concatenate(rows, axis=0).T.astype(np_mm))
        woT = np.ascontiguousarray(W_out[:, FD * c:FD * (c + 1)].T.astype(np_mm))
        in_maps.append({
            "xT": xT, "wqkvT": wqkvT, "woT": woT,
            "freq": freq, "madd": madd_dev, "caus": caus,
        })
    return in_maps


def _ensure_ntff_hook():
    """Install antenv.axon_hooks + the ctypes NTFF profile hook if the image
    lacks them (needed only for trace=True timing runs, not for kernel())."""
    import types
    try:
        from antenv.axon_hooks import get_axon_ntff_profile_hook  # noqa: F401
        return
    except ImportError:
        pass
    try:
        import antenv
        mod = types.ModuleType("antenv.axon_hooks")
        _state = {"hook": None}

        def set_axon_ntff_profile_hook(h):
            _state["hook"] = h

        def get_axon_ntff_profile_hook():
            return _state["hook"]

        mod.set_axon_ntff_profile_hook = set_axon_ntff_profile_hook
        mod.get_axon_ntff_profile_hook = get_axon_ntff_profile_hook
        sys.modules["antenv.axon_hooks"] = mod
        antenv.axon_hooks = mod
        from trn_agent_boot.trn_boot import _ntff_profile_via_ctypes
        hook = _ntff_profile_via_ctypes("/opt/axon/libaxon_pjrt.so")
        if hook is not None:
            set_axon_ntff_profile_hook(hook)
    except Exception as e:  # degrade to untimed runs
        print(f"ntff hook install failed: {e!r}", file=sys.stderr)


_NC_CACHE = {}


def _get_nc(dt_mode="f16"):
    if dt_mode not in _NC_CACHE:
        _NC_CACHE[dt_mode] = build_nc(dt_mode=dt_mode)
    return _NC_CACHE[dt_mode]


def run_cores(in_maps, trace=False, dt_mode="f16"):
    if trace:
        _ensure_ntff_hook()
    nc = _get_nc(dt_mode)
    res = run_bass_kernel_spmd(
        nc, in_maps, core_ids=list(range(NCORES)), trace=trace,
        trace_cores=list(range(NCORES)) if trace else None,
    )
    return res


DT_MODE = "f16"


def kernel(x, mask, rotary_pos_emb, W_qkv, W_out, b_out):
    in_maps = prep_inputs(x, mask, rotary_pos_emb, W_qkv, W_out, dt_mode=DT_MODE)
    res = run_cores(in_maps, trace=False, dt_mode=DT_MODE)
    y = np.zeros((NT, D), dtype=np.float32)
    for r in res.results:
        y += r["y"]
    y += np.asarray(b_out, dtype=np.float32)[None, :]
    return y.reshape(B, N, D)


# revision 18
# speedup vs baseline: 1.5227x; 1.1109x over previous
"""Trainium2 Bass kernel for nn_Attention_25383256719981.

Dense transformer attention block:
  qkv = x @ W_qkv.T ; rotary(q,k,v) ; causal+padding-masked softmax(q k^T / sqrt(dh)) @ v ;
  out = heads @ W_out.T + b_out

Sharding: tensor-parallel over heads. 16 heads / 8 cores = 2 heads per core.
Each core computes its 2 heads' QKV projection, attention, and a partial
output projection (y_partial = O_heads @ W_out[:, head_cols].T); the host
sums the 8 partials and adds b_out.

Device-side layout choices:
  - QKV matmul in "family B" layout: out[tokens(128p), 384 features] so the
    rotary pair-shuffle is a cheap strided access along the free axis.
  - q,k transposed on the PE to "family A" [dh, tokens] for the attention
    matmuls (contraction over dh must sit on partitions).
  - Attention computed transposed: S^T[k, q] = K Q^T, so the probability
    tile P^T = exp(...) feeds the O matmul directly as the moving operand.
  - V gets an appended ones column, so O'^T = [V|1]^T P^T yields both O^T
    and the softmax row-sums (Z) in one accumulation; normalization is a
    per-element multiply by broadcast 1/Z before the output projection.
  - Matmuls run as float32r (full-rate single-pass fp32 on the PE).
"""

import sys

import numpy as np

for _p in ("/opt/trn_rl_repo",):
    if _p not in sys.path:
        sys.path.insert(0, _p)

import concourse.bass as bass
import concourse.bacc as bacc
import concourse.mybir as mybir
import concourse.tile as tile
from concourse.bass_utils import run_bass_kernel_spmd
from concourse.masks import make_identity

# Problem shapes (hardcoded per contract).
B, N, D, H, DH = 2, 2048, 1024, 16, 64
NCORES = 8
HPC = H // NCORES            # heads per core
P = 128
NT = B * N                   # total tokens
SCALE = DH ** -0.5
FD = HPC * DH                # per-core features per tensor (128)
F3 = 3 * FD                  # 384
NEG = -1.0e30
NB = N // P                  # 16 token-blocks per batch
QC = N // 512                # 4 query chunks of 512 per batch

f32 = mybir.dt.float32
f32r = mybir.dt.float32r
f16 = mybir.dt.float16
AF = mybir.ActivationFunctionType
ALU = mybir.AluOpType


def _r(ap):
    """Bitcast an fp32 AP to float32r for full-rate PE matmuls."""
    return ap.bitcast(f32r)


def build_nc(dt_mode="f16"):
    nc = bacc.Bacc("TRN2", target_bir_lowering=False)
    fmm = {"f16": f16, "f32r": f32r, "f32": f32}[dt_mode]
    mm = lambda ap: ap

    xT = nc.dram_tensor("xT", [D, NT], fmm, kind="ExternalInput")
    wqkvT = nc.dram_tensor("wqkvT", [D, F3], fmm, kind="ExternalInput")
    woT = nc.dram_tensor("woT", [FD, D], fmm, kind="ExternalInput")
    freq = nc.dram_tensor("freq", [N, DH], f32, kind="ExternalInput")
    madd = nc.dram_tensor("madd", [P, B * NB], f32, kind="ExternalInput")
    caus = nc.dram_tensor("caus", [P, P], f32, kind="ExternalInput")
    y = nc.dram_tensor("y", [NT, D], f32, kind="ExternalOutput")

    xT_r = xT.rearrange("(ko p) t -> p ko t", p=P)          # [128, 8, 4096]
    wq_r = wqkvT.rearrange("(ko p) f -> p ko f", p=P)       # [128, 8, 384]
    freq_r = freq.rearrange("(t p) d -> p t d", p=P)        # [128, 16, 64]

    with tile.TileContext(nc) as tc, \
            tc.tile_pool(name="const", bufs=1) as const, \
            tc.tile_pool(name="xp", bufs=2) as xp, \
            tc.tile_pool(name="qkp", bufs=1) as qkp, \
            tc.tile_pool(name="vfp", bufs=2) as vfp, \
            tc.tile_pool(name="qtp", bufs=1) as qtp, \
            tc.tile_pool(name="tmpp", bufs=3) as tmpp, \
            tc.tile_pool(name="ptp", bufs=4) as ptp, \
            tc.tile_pool(name="plp", bufs=2) as plp, \
            tc.tile_pool(name="smallp", bufs=3) as smallp, \
            tc.tile_pool(name="yp", bufs=4) as yp, \
            tc.tile_pool(name="psmm", bufs=2, space="PSUM") as psmm, \
            tc.tile_pool(name="pss", bufs=2, space="PSUM") as pss, \
            tc.tile_pool(name="pstr", bufs=2, space="PSUM") as pstr, \
            tc.tile_pool(name="pso", bufs=2, space="PSUM") as pso:

        # ---- constants / weights ----
        w_sb = const.tile([P, D // P, F3], fmm, tag="w")
        nc.sync.dma_start(w_sb[:, :, :], wq_r)
        wo_sb = const.tile([FD, D], fmm, tag="wo")
        nc.sync.dma_start(wo_sb[:, :], woT[:, :])
        caus_sb = const.tile([P, P], f32, tag="caus")
        nc.sync.dma_start(caus_sb[:, :], caus[:, :])
        madd_sb = const.tile([P, B * NB], f32, tag="madd")
        nc.sync.dma_start(madd_sb[:, :], madd[:, :])
        freq_sb = const.tile([P, NB, DH], f32, tag="freq")
        nc.sync.dma_start(freq_sb[:, :, :], freq_r)
        ident_f32 = const.tile([P, P], f32, tag="ident_f32")
        make_identity(nc, ident_f32)
        ident = const.tile([P, P], fmm, tag="ident")
        nc.vector.tensor_copy(ident, ident_f32)
        onecol = const.tile([P, 1], f32, tag="onecol")
        nc.gpsimd.memset(onecol, 1.0)

        # cos = sin(wrap(freq + pi/2)); sin_signed: negated at even dh positions.
        # Scalar-engine Sin needs inputs in [-pi, pi]; add_range_wrap handles
        # the shift + one-period wrap (valid for |freq| < 3*pi - shift).
        PI = float(np.pi)

        def range_wrap(out, xs, scratch):
            # out = xs - 2*pi * ((xs > pi) - (xs < -pi)) : one-period wrap
            g = scratch.tile([P, NB, DH], f32, tag="wrap_g")
            lo = scratch.tile([P, NB, DH], f32, tag="wrap_l")
            del scratch
            nc.vector.tensor_scalar(g, xs, PI, None, ALU.is_gt)
            nc.vector.tensor_scalar(lo, xs, -PI, None, ALU.is_lt)
            nc.vector.tensor_tensor(g, g, lo, ALU.subtract)
            nc.vector.scalar_tensor_tensor(out, g, -2 * PI, xs, ALU.mult, ALU.add)

        wrap_s = const.tile([P, NB, DH], f32, tag="wrap_s")
        range_wrap(wrap_s, freq_sb, const)
        wrap_c = const.tile([P, NB, DH], f32, tag="wrap_c")
        shifted = const.tile([P, NB, DH], f32, tag="shifted")
        nc.vector.tensor_scalar(shifted, freq_sb, PI / 2, None, ALU.add)
        range_wrap(wrap_c, shifted, const)
        cos_sb = const.tile([P, NB, DH], f32, tag="cos")
        nc.scalar.activation(cos_sb, wrap_c, AF.Sin)
        sins_sb = const.tile([P, NB, DH], f32, tag="sins")
        nc.scalar.activation(sins_sb[:, :, 0::2], wrap_s[:, :, 0::2], AF.Sin,
                             scale=-1.0)
        nc.scalar.activation(sins_sb[:, :, 1::2], wrap_s[:, :, 1::2], AF.Sin,
                             scale=1.0)

        for b in range(B):
            # ================= Phase B: QKV projection + rotary ============
            qkB = qkp.tile([P, NB, 2 * FD], fmm, tag="qkB")     # q01|k01
            vfB = vfp.tile([P, NB, HPC * (DH + 1)], fmm, tag="vfB")
            # ones column at position 64 of each head's 65-wide group
            # (copy-cast from an f32 const: memset can't encode float32r)
            nc.vector.tensor_copy(vfB[:, :, DH::DH + 1],
                                  onecol[:, None, :].to_broadcast([P, NB, HPC]))

            for c in range(N // 512):                # 512-token chunks
                x_sb = xp.tile([P, D // P, 512], fmm, tag="x")
                tok0 = b * N + c * 512
                nc.sync.dma_start(x_sb[:, :, :], xT_r[:, :, tok0:tok0 + 512])
                for tb in range(4):
                    t = c * 4 + tb                   # token-block in batch
                    qkv_ps = psmm.tile([P, F3], f32, tag="mm")
                    for ko in range(D // P):
                        nc.tensor.matmul(
                            qkv_ps,
                            mm(x_sb[:, ko, tb * P:(tb + 1) * P]),
                            mm(w_sb[:, ko, :]),
                            start=(ko == 0), stop=(ko == D // P - 1),
                        )
                    ps_g = qkv_ps.rearrange("p (g d) -> p g d", g=6)
                    # pass 1: tmp = pairswap(qkv) * sin_signed
                    tmp = tmpp.tile([P, F3], f32, tag="tmp")
                    tmp_g = tmp.rearrange("p (g d) -> p g d", g=6)
                    se = sins_sb[:, t, 0::2][:, None, :].to_broadcast([P, 6, DH // 2])
                    so = sins_sb[:, t, 1::2][:, None, :].to_broadcast([P, 6, DH // 2])
                    nc.vector.tensor_tensor(tmp_g[:, :, 0::2], ps_g[:, :, 1::2], se, ALU.mult)
                    nc.vector.tensor_tensor(tmp_g[:, :, 1::2], ps_g[:, :, 0::2], so, ALU.mult)
                    # pass 2: cosq = qkv * cos
                    cosq = tmpp.tile([P, F3], f32, tag="cosq")
                    cosq_g = cosq.rearrange("p (g d) -> p g d", g=6)
                    cb = cos_sb[:, t, :][:, None, :].to_broadcast([P, 6, DH])
                    nc.vector.tensor_tensor(cosq_g, ps_g, cb, ALU.mult)
                    # pass 3: rotated = tmp + cosq (q,k -> qkB; v -> vfB)
                    nc.gpsimd.tensor_tensor(qkB[:, t, :], tmp[:, 0:2 * FD],
                                            cosq[:, 0:2 * FD], ALU.add)
                    vf_v = vfB[:, t, :].rearrange("p (h c) -> p h c", h=HPC)[:, :, 0:DH]
                    tmp_v = tmp[:, 2 * FD:F3].rearrange("p (h d) -> p h d", h=HPC)
                    cos_v = cosq[:, 2 * FD:F3].rearrange("p (h d) -> p h d", h=HPC)
                    nc.gpsimd.tensor_tensor(vf_v, tmp_v, cos_v, ALU.add)

            # ================= Phase C: transpose q,k to [dh, tok] =========
            # Q is split into per-head tiles zero-padded to 128 contraction
            # rows, so the S matmul can use the full 2-head K block as lhsT
            # with K=128 (K=64 matmuls run the PE at half rate).
            QT0 = qtp.tile([P, N], fmm, tag="QT0")
            QT1 = qtp.tile([P, N], fmm, tag="QT1")
            KT = qtp.tile([P, N], fmm, tag="KT")
            nc.gpsimd.memset(QT0[DH:P, :], 0.0)
            nc.gpsimd.memset(QT1[0:DH, :], 0.0)
            for t in range(NB):
                for which in (0, 1):
                    src = qkB[:, t, which * FD:(which + 1) * FD]
                    tr_ps = pstr.tile([P, P], fmm, tag="tr")
                    nc.tensor.transpose(tr_ps, src, ident)
                    if which == 0:
                        nc.vector.tensor_copy(QT0[0:DH, t * P:(t + 1) * P],
                                              tr_ps[0:DH, :])
                        nc.scalar.copy(QT1[DH:P, t * P:(t + 1) * P],
                                       tr_ps[DH:P, :])
                    elif t % 2 == 0:
                        nc.vector.tensor_copy(KT[:, t * P:(t + 1) * P], tr_ps)
                    else:
                        nc.scalar.copy(KT[:, t * P:(t + 1) * P], tr_ps)

            # ================= Phase D: attention ==========================
            # Software-pipelined: the S matmul for chunk i+1 is emitted before
            # exp/O of chunk i so the PE never stalls on the ScalarE exp.
            PL = plp.tile([P, N], fmm, tag="PL")    # normalized O^T, 2 heads
            O_tiles = {}
            QTs = (QT0, QT1)

            def emit_S(qc, kb, h):
                qs = max(kb * P, 512 * qc)
                off = qs - 512 * qc
                w = 512 - off
                S_t = pss.tile([P, 512], f32, tag="s")
                nc.tensor.matmul(S_t[:, :w], mm(KT[:, kb * P:(kb + 1) * P]),
                                 mm(QTs[h][:, qs:qs + w]), start=True, stop=True)
                if kb >= 4 * qc:  # chunk starts at the diagonal block
                    nc.vector.tensor_tensor(S_t[:, 0:P], S_t[:, 0:P],
                                            caus_sb, ALU.add)
                return (qc, kb, h, off, w, S_t)

            def emit_expO(qc, kb, h, off, w, S_t, b=b):
                pt = ptp.tile([P, 512], fmm, tag="pt")
                col = b * NB + kb
                nc.scalar.activation(pt[:, :w], S_t[:, :w], AF.Exp,
                                     bias=madd_sb[:, col:col + 1],
                                     scale=SCALE)
                if kb == 0:
                    O_tiles[(h, qc)] = pso.tile([DH + 1, 512], f32, tag="o",
                                                name=f"O_{h}_{qc}")
                O_ps = O_tiles[(h, qc)]
                nc.tensor.matmul(
                    O_ps[:, off:512],
                    mm(vfB[:, kb, h * (DH + 1):(h + 1) * (DH + 1)]),
                    mm(pt[:, :w]),
                    start=(kb == 0), stop=(kb == 4 * qc + 3),
                )
                if kb == 4 * qc + 3:
                    # normalize: PL[h] = O^T * broadcast(1/Z).  1/Z via a fold
                    # to [128, 4] so the DVE reciprocal runs on all lanes.
                    zrow = smallp.tile([1, 512], f32, tag="zrow")
                    nc.scalar.copy(zrow, O_ps[DH:DH + 1, :])
                    zf = smallp.tile([P, 4], f32, tag="zf")
                    nc.sync.dma_start(
                        zf[:, :], zrow.rearrange("o (p c) -> o p c", p=P))
                    rf = smallp.tile([P, 4], f32, tag="rf")
                    nc.vector.reciprocal(rf, zf)
                    rrow = smallp.tile([1, 512], f32, tag="rrow")
                    nc.sync.dma_start(
                        rrow.rearrange("o (p c) -> o p c", p=P), rf[:, :])
                    rb = smallp.tile([DH, 512], f32, tag="rb")
                    nc.gpsimd.partition_broadcast(rb, rrow)
                    nc.vector.tensor_tensor(
                        PL[h * DH:(h + 1) * DH, 512 * qc:512 * (qc + 1)],
                        O_ps[0:DH, :], rb, ALU.mult)

            units = [(qc, kb, h) for qc in range(QC)
                     for kb in range(4 * qc + 4) for h in range(HPC)]
            pend = None
            for u in units:
                cur = emit_S(*u)
                if pend is not None:
                    emit_expO(*pend)
                pend = cur
            emit_expO(*pend)

            # ================= Phase E: output projection ==================
            for t in range(NB):
                y_sb = yp.tile([P, D], f32, tag="ysb")
                for dc in range(2):
                    y_ps = psmm.tile([P, 512], f32, tag="mm")
                    nc.tensor.matmul(y_ps, mm(PL[:, t * P:(t + 1) * P]),
                                     mm(wo_sb[:, dc * 512:(dc + 1) * 512]),
                                     start=True, stop=True)
                    if dc == 0:
                        nc.vector.tensor_copy(y_sb[:, 0:512], y_ps)
                    else:
                        nc.scalar.copy(y_sb[:, 512:1024], y_ps)
                r0 = b * N + t * P
                nc.sync.dma_start(y[r0:r0 + P, :], y_sb)

    nc.compile()
    return nc


def prep_inputs(x, mask, rotary_pos_emb, W_qkv, W_out, dt_mode="f16"):
    """Host-side shard prep: per-core input dicts (layout only + mask encode)."""
    np_mm = np.float16 if dt_mode == "f16" else np.float32
    x = np.asarray(x, dtype=np.float32)
    W_qkv = np.asarray(W_qkv, dtype=np.float32)
    W_out = np.asarray(W_out, dtype=np.float32)
    rope = np.asarray(rotary_pos_emb, dtype=np.float32)
    mask = np.asarray(mask)

    xT = np.ascontiguousarray(x.reshape(NT, D).T.astype(np_mm))
    madd = np.where(mask, np.float32(0.0), np.float32(NEG)).astype(np.float32)
    madd_dev = np.ascontiguousarray(
        madd.reshape(B, NB, P).transpose(2, 0, 1).reshape(P, B * NB))
    kidx = np.arange(P)[:, None]
    qidx = np.arange(P)[None, :]
    caus = np.where(qidx >= kidx, np.float32(0.0), np.float32(NEG)).astype(np.float32)
    freq = np.ascontiguousarray(rope[-N:, :])

    in_maps = []
    for c in range(NCORES):
        rows = []
        for tsel in range(3):                      # q, k, v row blocks
            for h in (HPC * c, HPC * c + 1):
                o = tsel * H * DH + h * DH
                rows.append(W_qkv[o:o + DH, :])
        wqkvT = np.ascontiguousarray(np.concatenate(rows, axis=0).T.astype(np_mm))
        woT = np.ascontiguousarray(W_out[:, FD * c:FD * (c + 1)].T.astype(np_mm))
        in_maps.append({
            "xT": xT, "wqkvT": wqkvT, "woT": woT,
            "freq": freq, "madd": madd_dev, "caus": caus,
        })
    return in_maps


def _ensure_ntff_hook():
    """Install antenv.axon_hooks + the ctypes NTFF profile hook if the image
    lacks them (needed only for trace=True timing runs, not for kernel())."""
    import types
    try:
        from antenv.axon_hooks import get_axon_ntff_profile_hook  # noqa: F401
        return
    except ImportError:
        pass
    try:
        import antenv
        mod = types.ModuleType("antenv.axon_hooks")
        _state = {"hook": None}

        def set_axon_ntff_profile_hook(h):
            _state["hook"] = h

        def get_axon_ntff_profile_hook():
            return _state["hook"]

        mod.set_axon_ntff_profile_hook = set_axon_ntff_profile_hook
        mod.get_axon_ntff_profile_hook = get_axon_ntff_profile_hook
        sys.modules["antenv.axon_hooks"] = mod
        antenv.axon_hooks = mod
        from trn_agent_boot.trn_boot import _ntff_profile_via_ctypes
        hook = _ntff_profile_via_ctypes("/opt/axon/libaxon_pjrt.so")
        if hook is not None:
            set_axon_ntff_profile_hook(hook)
    except Exception as e:  # degrade to untimed runs
        print(f"ntff hook install failed: {e!r}", file=sys.stderr)


_NC_CACHE = {}


def _get_nc(dt_mode="f16"):
    if dt_mode not in _NC_CACHE:
        _NC_CACHE[dt_mode] = build_nc(dt_mode=dt_mode)
    return _NC_CACHE[dt_mode]


def run_cores(in_maps, trace=False, dt_mode="f16"):
    if trace:
        _ensure_ntff_hook()
    nc = _get_nc(dt_mode)
    res = run_bass_kernel_spmd(
        nc, in_maps, core_ids=list(range(NCORES)), trace=trace,
        trace_cores=list(range(NCORES)) if trace else None,
    )
    return res


DT_MODE = "f16"


def kernel(x, mask, rotary_pos_emb, W_qkv, W_out, b_out):
    in_maps = prep_inputs(x, mask, rotary_pos_emb, W_qkv, W_out, dt_mode=DT_MODE)
    res = run_cores(in_maps, trace=False, dt_mode=DT_MODE)
    y = np.zeros((NT, D), dtype=np.float32)
    for r in res.results:
        y += r["y"]
    y += np.asarray(b_out, dtype=np.float32)[None, :]
    return y.reshape(B, N, D)


# revision 21
# speedup vs baseline: 1.5899x; 1.0441x over previous
"""Trainium2 Bass kernel for nn_Attention_25383256719981.

Dense transformer attention block:
  qkv = x @ W_qkv.T ; rotary(q,k,v) ; causal+padding-masked softmax(q k^T / sqrt(dh)) @ v ;
  out = heads @ W_out.T + b_out

Sharding: tensor-parallel over heads. 16 heads / 8 cores = 2 heads per core.
Each core computes its 2 heads' QKV projection, attention, and a partial
output projection (y_partial = O_heads @ W_out[:, head_cols].T); the host
sums the 8 partials and adds b_out.

Device-side layout choices:
  - QKV matmul in "family B" layout: out[tokens(128p), 384 features] so the
    rotary pair-shuffle is a cheap strided access along the free axis.
  - q,k transposed on the PE to "family A" [dh, tokens] for the attention
    matmuls (contraction over dh must sit on partitions).
  - Attention computed transposed: S^T[k, q] = K Q^T, so the probability
    tile P^T = exp(...) feeds the O matmul directly as the moving operand.
  - V gets an appended ones column, so O'^T = [V|1]^T P^T yields both O^T
    and the softmax row-sums (Z) in one accumulation; normalization is a
    per-element multiply by broadcast 1/Z before the output projection.
  - Matmuls run as float32r (full-rate single-pass fp32 on the PE).
"""

import sys

import numpy as np

for _p in ("/opt/trn_rl_repo",):
    if _p not in sys.path:
        sys.path.insert(0, _p)

import concourse.bass as bass
import concourse.bacc as bacc
import concourse.mybir as mybir
import concourse.tile as tile
from concourse.bass_utils import run_bass_kernel_spmd
from concourse.masks import make_identity

# Problem shapes (hardcoded per contract).
B, N, D, H, DH = 2, 2048, 1024, 16, 64
NCORES = 8
HPC = H // NCORES            # heads per core
P = 128
NT = B * N                   # total tokens
SCALE = DH ** -0.5
FD = HPC * DH                # per-core features per tensor (128)
F3 = 3 * FD                  # 384
NEG = -1.0e30
NB = N // P                  # 16 token-blocks per batch
QC = N // 512                # 4 query chunks of 512 per batch

f32 = mybir.dt.float32
f32r = mybir.dt.float32r
f16 = mybir.dt.float16
AF = mybir.ActivationFunctionType
ALU = mybir.AluOpType


def _r(ap):
    """Bitcast an fp32 AP to float32r for full-rate PE matmuls."""
    return ap.bitcast(f32r)


def build_nc(dt_mode="f16"):
    nc = bacc.Bacc("TRN2", target_bir_lowering=False)
    fmm = {"f16": f16, "f32r": f32r, "f32": f32}[dt_mode]
    mm = lambda ap: ap

    xT = nc.dram_tensor("xT", [D, NT], fmm, kind="ExternalInput")
    wqkvT = nc.dram_tensor("wqkvT", [D, F3], fmm, kind="ExternalInput")
    woT = nc.dram_tensor("woT", [FD, D], fmm, kind="ExternalInput")
    freq = nc.dram_tensor("freq", [N, DH], f32, kind="ExternalInput")
    madd = nc.dram_tensor("madd", [P, B * NB], f32, kind="ExternalInput")
    caus = nc.dram_tensor("caus", [P, P], f32, kind="ExternalInput")
    y = nc.dram_tensor("y", [NT, D], f32, kind="ExternalOutput")

    xT_r = xT.rearrange("(ko p) t -> p ko t", p=P)          # [128, 8, 4096]
    wq_r = wqkvT.rearrange("(ko p) f -> p ko f", p=P)       # [128, 8, 384]
    freq_r = freq.rearrange("(t p) d -> p t d", p=P)        # [128, 16, 64]

    with tile.TileContext(nc) as tc, \
            tc.tile_pool(name="const", bufs=1) as const, \
            tc.tile_pool(name="xp", bufs=2) as xp, \
            tc.tile_pool(name="qkp", bufs=1) as qkp, \
            tc.tile_pool(name="vfp", bufs=2) as vfp, \
            tc.tile_pool(name="qtp", bufs=1) as qtp, \
            tc.tile_pool(name="tmpp", bufs=3) as tmpp, \
            tc.tile_pool(name="ptp", bufs=6) as ptp, \
            tc.tile_pool(name="plp", bufs=2) as plp, \
            tc.tile_pool(name="smallp", bufs=3) as smallp, \
            tc.tile_pool(name="yp", bufs=4) as yp, \
            tc.tile_pool(name="psmm", bufs=2, space="PSUM") as psmm, \
            tc.tile_pool(name="pss", bufs=2, space="PSUM") as pss, \
            tc.tile_pool(name="pstr", bufs=2, space="PSUM") as pstr, \
            tc.tile_pool(name="pso", bufs=2, space="PSUM") as pso:

        # ---- constants / weights ----
        w_sb = const.tile([P, D // P, F3], fmm, tag="w")
        nc.sync.dma_start(w_sb[:, :, :], wq_r)
        wo_sb = const.tile([FD, D], fmm, tag="wo")
        nc.sync.dma_start(wo_sb[:, :], woT[:, :])
        caus_sb = const.tile([P, P], f32, tag="caus")
        nc.sync.dma_start(caus_sb[:, :], caus[:, :])
        # 0/1 multiplicative causal mask in matmul dtype (applied post-exp)
        caus01 = const.tile([P, P], fmm, tag="caus01")
        nc.vector.tensor_scalar(caus01, caus_sb, -0.5, None, ALU.is_ge)
        madd_sb = const.tile([P, B * NB], f32, tag="madd")
        nc.sync.dma_start(madd_sb[:, :], madd[:, :])
        freq_sb = const.tile([P, NB, DH], f32, tag="freq")
        nc.sync.dma_start(freq_sb[:, :, :], freq_r)
        ident_f32 = const.tile([P, P], f32, tag="ident_f32")
        make_identity(nc, ident_f32)
        ident = const.tile([P, P], fmm, tag="ident")
        nc.vector.tensor_copy(ident, ident_f32)
        onecol = const.tile([P, 1], f32, tag="onecol")
        nc.gpsimd.memset(onecol, 1.0)

        # cos = sin(wrap(freq + pi/2)); sin_signed: negated at even dh positions.
        # Scalar-engine Sin needs inputs in [-pi, pi]; add_range_wrap handles
        # the shift + one-period wrap (valid for |freq| < 3*pi - shift).
        PI = float(np.pi)

        def range_wrap(out, xs, scratch):
            # out = xs - 2*pi * ((xs > pi) - (xs < -pi)) : one-period wrap
            g = scratch.tile([P, NB, DH], f32, tag="wrap_g")
            lo = scratch.tile([P, NB, DH], f32, tag="wrap_l")
            del scratch
            nc.vector.tensor_scalar(g, xs, PI, None, ALU.is_gt)
            nc.vector.tensor_scalar(lo, xs, -PI, None, ALU.is_lt)
            nc.vector.tensor_tensor(g, g, lo, ALU.subtract)
            nc.vector.scalar_tensor_tensor(out, g, -2 * PI, xs, ALU.mult, ALU.add)

        wrap_s = const.tile([P, NB, DH], f32, tag="wrap_s")
        range_wrap(wrap_s, freq_sb, const)
        wrap_c = const.tile([P, NB, DH], f32, tag="wrap_c")
        shifted = const.tile([P, NB, DH], f32, tag="shifted")
        nc.vector.tensor_scalar(shifted, freq_sb, PI / 2, None, ALU.add)
        range_wrap(wrap_c, shifted, const)
        cos_sb = const.tile([P, NB, DH], f32, tag="cos")
        nc.scalar.activation(cos_sb, wrap_c, AF.Sin)
        sins_sb = const.tile([P, NB, DH], f32, tag="sins")
        nc.scalar.activation(sins_sb[:, :, 0::2], wrap_s[:, :, 0::2], AF.Sin,
                             scale=-1.0)
        nc.scalar.activation(sins_sb[:, :, 1::2], wrap_s[:, :, 1::2], AF.Sin,
                             scale=1.0)

        for b in range(B):
            # ================= Phase B: QKV projection + rotary ============
            qkB = qkp.tile([P, NB, 2 * FD], fmm, tag="qkB")     # q01|k01
            vfB = vfp.tile([P, NB, HPC * (DH + 1)], fmm, tag="vfB")
            # ones column at position 64 of each head's 65-wide group
            # (copy-cast from an f32 const: memset can't encode float32r)
            nc.vector.tensor_copy(vfB[:, :, DH::DH + 1],
                                  onecol[:, None, :].to_broadcast([P, NB, HPC]))

            for c in range(N // 512):                # 512-token chunks
                x_sb = xp.tile([P, D // P, 512], fmm, tag="x")
                tok0 = b * N + c * 512
                nc.sync.dma_start(x_sb[:, :, :], xT_r[:, :, tok0:tok0 + 512])
                for tb in range(4):
                    t = c * 4 + tb                   # token-block in batch
                    qkv_ps = psmm.tile([P, F3], f32, tag="mm")
                    for ko in range(D // P):
                        nc.tensor.matmul(
                            qkv_ps,
                            mm(x_sb[:, ko, tb * P:(tb + 1) * P]),
                            mm(w_sb[:, ko, :]),
                            start=(ko == 0), stop=(ko == D // P - 1),
                        )
                    ps_g = qkv_ps.rearrange("p (g d) -> p g d", g=6)
                    # pass 1: tmp = pairswap(qkv) * sin_signed
                    tmp = tmpp.tile([P, F3], f32, tag="tmp")
                    tmp_g = tmp.rearrange("p (g d) -> p g d", g=6)
                    se = sins_sb[:, t, 0::2][:, None, :].to_broadcast([P, 6, DH // 2])
                    so = sins_sb[:, t, 1::2][:, None, :].to_broadcast([P, 6, DH // 2])
                    nc.vector.tensor_tensor(tmp_g[:, :, 0::2], ps_g[:, :, 1::2], se, ALU.mult)
                    nc.vector.tensor_tensor(tmp_g[:, :, 1::2], ps_g[:, :, 0::2], so, ALU.mult)
                    # pass 2: cosq = qkv * cos
                    cosq = tmpp.tile([P, F3], f32, tag="cosq")
                    cosq_g = cosq.rearrange("p (g d) -> p g d", g=6)
                    cb = cos_sb[:, t, :][:, None, :].to_broadcast([P, 6, DH])
                    nc.vector.tensor_tensor(cosq_g, ps_g, cb, ALU.mult)
                    # pass 3: rotated = tmp + cosq (q,k -> qkB; v -> vfB)
                    nc.gpsimd.tensor_tensor(qkB[:, t, :], tmp[:, 0:2 * FD],
                                            cosq[:, 0:2 * FD], ALU.add)
                    vf_v = vfB[:, t, :].rearrange("p (h c) -> p h c", h=HPC)[:, :, 0:DH]
                    tmp_v = tmp[:, 2 * FD:F3].rearrange("p (h d) -> p h d", h=HPC)
                    cos_v = cosq[:, 2 * FD:F3].rearrange("p (h d) -> p h d", h=HPC)
                    nc.gpsimd.tensor_tensor(vf_v, tmp_v, cos_v, ALU.add)

            # ================= Phase C: transpose q,k to [dh, tok] =========
            # Q is split into per-head tiles zero-padded to 128 contraction
            # rows, so the S matmul can use the full 2-head K block as lhsT
            # with K=128 (K=64 matmuls run the PE at half rate).
            QT0 = qtp.tile([P, N], fmm, tag="QT0")
            QT1 = qtp.tile([P, N], fmm, tag="QT1")
            KT = qtp.tile([P, N], fmm, tag="KT")
            nc.gpsimd.memset(QT0[DH:P, :], 0.0)
            nc.gpsimd.memset(QT1[0:DH, :], 0.0)
            for t in range(NB):
                for which in (0, 1):
                    src = qkB[:, t, which * FD:(which + 1) * FD]
                    tr_ps = pstr.tile([P, P], fmm, tag="tr")
                    nc.tensor.transpose(tr_ps, src, ident)
                    if which == 0:
                        nc.vector.tensor_copy(QT0[0:DH, t * P:(t + 1) * P],
                                              tr_ps[0:DH, :])
                        nc.scalar.copy(QT1[DH:P, t * P:(t + 1) * P],
                                       tr_ps[DH:P, :])
                    elif t % 2 == 0:
                        nc.vector.tensor_copy(KT[:, t * P:(t + 1) * P], tr_ps)
                    else:
                        nc.scalar.copy(KT[:, t * P:(t + 1) * P], tr_ps)

            # ================= Phase D: attention ==========================
            # Software-pipelined: the S matmul for chunk i+1 is emitted before
            # exp/O of chunk i so the PE never stalls on the ScalarE exp.
            O_tiles = {}
            PL_tiles = {}
            QTs = (QT0, QT1)

            def emit_proj(qc, b=b):
                PLq = PL_tiles.pop(qc)
                for tb in range(4):
                    t = qc * 4 + tb
                    y_sb = yp.tile([P, D], f32, tag="ysb")
                    for dc in range(2):
                        y_ps = psmm.tile([P, 512], f32, tag="mm", name=f"yps{dc}")
                        nc.tensor.matmul(y_ps,
                                         mm(PLq[:, tb * P:(tb + 1) * P]),
                                         mm(wo_sb[:, dc * 512:(dc + 1) * 512]),
                                         start=True, stop=True)
                        if dc == 0:
                            nc.vector.tensor_copy(y_sb[:, 0:512], y_ps)
                        else:
                            nc.scalar.copy(y_sb[:, 512:1024], y_ps)
                    r0 = b * N + t * P
                    nc.sync.dma_start(y[r0:r0 + P, :], y_sb)

            def emit_S(qc, kb, h):
                qs = max(kb * P, 512 * qc)
                off = qs - 512 * qc
                w = 512 - off
                S_t = pss.tile([P, 512], f32, tag="s")
                nc.tensor.matmul(S_t[:, :w], mm(KT[:, kb * P:(kb + 1) * P]),
                                 mm(QTs[h][:, qs:qs + w]), start=True, stop=True)
                return (qc, kb, h, off, w, S_t)

            def emit_expO(qc, kb, h, off, w, S_t, b=b):
                pt = ptp.tile([P, 512], fmm, tag="pt")
                col = b * NB + kb
                nc.scalar.activation(pt[:, :w], S_t[:, :w], AF.Exp,
                                     bias=madd_sb[:, col:col + 1],
                                     scale=SCALE)
                if kb >= 4 * qc:  # chunk starts at the diagonal block
                    nc.vector.tensor_tensor(pt[:, 0:P], pt[:, 0:P], caus01,
                                            ALU.mult)
                if kb == 0:
                    O_tiles[(h, qc)] = pso.tile([DH + 1, 512], f32, tag="o",
                                                name=f"O_{h}_{qc}")
                O_ps = O_tiles[(h, qc)]
                nc.tensor.matmul(
                    O_ps[:, off:512],
                    mm(vfB[:, kb, h * (DH + 1):(h + 1) * (DH + 1)]),
                    mm(pt[:, :w]),
                    start=(kb == 0), stop=(kb == 4 * qc + 3),
                )
                if kb == 4 * qc + 3:
                    # normalize: PL[h] = O^T * broadcast(1/Z).  1/Z via a fold
                    # to [128, 4] so the DVE reciprocal runs on all lanes.
                    zrow = smallp.tile([1, 512], f32, tag="zrow")
                    nc.scalar.copy(zrow, O_ps[DH:DH + 1, :])
                    zf = smallp.tile([P, 4], f32, tag="zf")
                    nc.sync.dma_start(
                        zf[:, :], zrow.rearrange("o (p c) -> o p c", p=P))
                    rf = smallp.tile([P, 4], f32, tag="rf")
                    nc.vector.reciprocal(rf, zf)
                    rrow = smallp.tile([1, 512], f32, tag="rrow")
                    nc.sync.dma_start(
                        rrow.rearrange("o (p c) -> o p c", p=P), rf[:, :])
                    rb = smallp.tile([DH, 512], f32, tag="rb")
                    nc.gpsimd.partition_broadcast(rb, rrow)
                    if qc not in PL_tiles:
                        PL_tiles[qc] = plp.tile([P, 512], fmm, tag="PL",
                                                name=f"PL_{qc}")
                    nc.vector.tensor_tensor(
                        PL_tiles[qc][h * DH:(h + 1) * DH, :],
                        O_ps[0:DH, :], rb, ALU.mult)
                    if h == HPC - 1:
                        emit_proj(qc)

            units = [(qc, kb, h) for qc in range(QC)
                     for kb in range(4 * qc + 4) for h in range(HPC)]
            pend = None
            for u in units:
                cur = emit_S(*u)
                if pend is not None:
                    emit_expO(*pend)
                pend = cur
            emit_expO(*pend)


    nc.compile()
    return nc


def prep_inputs(x, mask, rotary_pos_emb, W_qkv, W_out, dt_mode="f16"):
    """Host-side shard prep: per-core input dicts (layout only + mask encode)."""
    np_mm = np.float16 if dt_mode == "f16" else np.float32
    x = np.asarray(x, dtype=np.float32)
    W_qkv = np.asarray(W_qkv, dtype=np.float32)
    W_out = np.asarray(W_out, dtype=np.float32)
    rope = np.asarray(rotary_pos_emb, dtype=np.float32)
    mask = np.asarray(mask)

    xT = np.ascontiguousarray(x.reshape(NT, D).T.astype(np_mm))
    madd = np.where(mask, np.float32(0.0), np.float32(NEG)).astype(np.float32)
    madd_dev = np.ascontiguousarray(
        madd.reshape(B, NB, P).transpose(2, 0, 1).reshape(P, B * NB))
    kidx = np.arange(P)[:, None]
    qidx = np.arange(P)[None, :]
    caus = np.where(qidx >= kidx, np.float32(0.0), np.float32(NEG)).astype(np.float32)
    freq = np.ascontiguousarray(rope[-N:, :])

    in_maps = []
    for c in range(NCORES):
        rows = []
        for tsel in range(3):                      # q, k, v row blocks
            for h in (HPC * c, HPC * c + 1):
                o = tsel * H * DH + h * DH
                rows.append(W_qkv[o:o + DH, :])
        wqkvT = np.ascontiguousarray(np.concatenate(rows, axis=0).T.astype(np_mm))
        woT = np.ascontiguousarray(W_out[:, FD * c:FD * (c + 1)].T.astype(np_mm))
        in_maps.append({
            "xT": xT, "wqkvT": wqkvT, "woT": woT,
            "freq": freq, "madd": madd_dev, "caus": caus,
        })
    return in_maps


def _ensure_ntff_hook():
    """Install antenv.axon_hooks + the ctypes NTFF profile hook if the image
    lacks them (needed only for trace=True timing runs, not for kernel())."""
    import types
    try:
        from antenv.axon_hooks import get_axon_ntff_profile_hook  # noqa: F401
        return
    except ImportError:
        pass
    try:
        import antenv
        mod = types.ModuleType("antenv.axon_hooks")
        _state = {"hook": None}

        def set_axon_ntff_profile_hook(h):
            _state["hook"] = h

        def get_axon_ntff_profile_hook():
            return _state["hook"]

        mod.set_axon_ntff_profile_hook = set_axon_ntff_profile_hook
        mod.get_axon_ntff_profile_hook = get_axon_ntff_profile_hook
        sys.modules["antenv.axon_hooks"] = mod
        antenv.axon_hooks = mod
        from trn_agent_boot.trn_boot import _ntff_profile_via_ctypes
        hook = _ntff_profile_via_ctypes("/opt/axon/libaxon_pjrt.so")
        if hook is not None:
            set_axon_ntff_profile_hook(hook)
    except Exception as e:  # degrade to untimed runs
        print(f"ntff hook install failed: {e!r}", file=sys.stderr)


_NC_CACHE = {}


def _get_nc(dt_mode="f16"):
    if dt_mode not in _NC_CACHE:
        _NC_CACHE[dt_mode] = build_nc(dt_mode=dt_mode)
    return _NC_CACHE[dt_mode]


def run_cores(in_maps, trace=False, dt_mode="f16"):
    if trace:
        _ensure_ntff_hook()
    nc = _get_nc(dt_mode)
    res = run_bass_kernel_spmd(
        nc, in_maps, core_ids=list(range(NCORES)), trace=trace,
        trace_cores=list(range(NCORES)) if trace else None,
    )
    return res


DT_MODE = "f16"


def kernel(x, mask, rotary_pos_emb, W_qkv, W_out, b_out):
    in_maps = prep_inputs(x, mask, rotary_pos_emb, W_qkv, W_out, dt_mode=DT_MODE)
    res = run_cores(in_maps, trace=False, dt_mode=DT_MODE)
    y = np.zeros((NT, D), dtype=np.float32)
    for r in res.results:
        y += r["y"]
    y += np.asarray(b_out, dtype=np.float32)[None, :]
    return y.reshape(B, N, D)


# revision 22
# speedup vs baseline: 1.5928x; 1.0019x over previous
"""Trainium2 Bass kernel for nn_Attention_25383256719981.

Dense transformer attention block:
  qkv = x @ W_qkv.T ; rotary(q,k,v) ; causal+padding-masked softmax(q k^T / sqrt(dh)) @ v ;
  out = heads @ W_out.T + b_out

Sharding: tensor-parallel over heads. 16 heads / 8 cores = 2 heads per core.
Each core computes its 2 heads' QKV projection, attention, and a partial
output projection (y_partial = O_heads @ W_out[:, head_cols].T); the host
sums the 8 partials and adds b_out.

Device-side layout choices:
  - QKV matmul in "family B" layout: out[tokens(128p), 384 features] so the
    rotary pair-shuffle is a cheap strided access along the free axis.
  - q,k transposed on the PE to "family A" [dh, tokens] for the attention
    matmuls (contraction over dh must sit on partitions).
  - Attention computed transposed: S^T[k, q] = K Q^T, so the probability
    tile P^T = exp(...) feeds the O matmul directly as the moving operand.
  - V gets an appended ones column, so O'^T = [V|1]^T P^T yields both O^T
    and the softmax row-sums (Z) in one accumulation; normalization is a
    per-element multiply by broadcast 1/Z before the output projection.
  - Matmuls run as float32r (full-rate single-pass fp32 on the PE).
"""

import sys

import numpy as np

for _p in ("/opt/trn_rl_repo",):
    if _p not in sys.path:
        sys.path.insert(0, _p)

import concourse.bass as bass
import concourse.bacc as bacc
import concourse.mybir as mybir
import concourse.tile as tile
from concourse.bass_utils import run_bass_kernel_spmd
from concourse.masks import make_identity

# Problem shapes (hardcoded per contract).
B, N, D, H, DH = 2, 2048, 1024, 16, 64
NCORES = 8
HPC = H // NCORES            # heads per core
P = 128
NT = B * N                   # total tokens
SCALE = DH ** -0.5
FD = HPC * DH                # per-core features per tensor (128)
F3 = 3 * FD                  # 384
NEG = -1.0e30
NB = N // P                  # 16 token-blocks per batch
QC = N // 512                # 4 query chunks of 512 per batch

f32 = mybir.dt.float32
f32r = mybir.dt.float32r
f16 = mybir.dt.float16
AF = mybir.ActivationFunctionType
ALU = mybir.AluOpType


def _r(ap):
    """Bitcast an fp32 AP to float32r for full-rate PE matmuls."""
    return ap.bitcast(f32r)


def build_nc(dt_mode="f16"):
    nc = bacc.Bacc("TRN2", target_bir_lowering=False)
    fmm = {"f16": f16, "f32r": f32r, "f32": f32}[dt_mode]
    mm = lambda ap: ap

    xT = nc.dram_tensor("xT", [D, NT], fmm, kind="ExternalInput")
    wqkvT = nc.dram_tensor("wqkvT", [D, F3], fmm, kind="ExternalInput")
    woT = nc.dram_tensor("woT", [FD, D], fmm, kind="ExternalInput")
    freq = nc.dram_tensor("freq", [N, DH], f32, kind="ExternalInput")
    madd = nc.dram_tensor("madd", [P, B * NB], f32, kind="ExternalInput")
    caus = nc.dram_tensor("caus", [P, P], f32, kind="ExternalInput")
    y = nc.dram_tensor("y", [NT, D], f32, kind="ExternalOutput")

    xT_r = xT.rearrange("(ko p) t -> p ko t", p=P)          # [128, 8, 4096]
    wq_r = wqkvT.rearrange("(ko p) f -> p ko f", p=P)       # [128, 8, 384]
    freq_r = freq.rearrange("(t p) d -> p t d", p=P)        # [128, 16, 64]

    with tile.TileContext(nc) as tc, \
            tc.tile_pool(name="const", bufs=1) as const, \
            tc.tile_pool(name="xp", bufs=3) as xp, \
            tc.tile_pool(name="qkp", bufs=1) as qkp, \
            tc.tile_pool(name="vfp", bufs=2) as vfp, \
            tc.tile_pool(name="qtp", bufs=1) as qtp, \
            tc.tile_pool(name="tmpp", bufs=4) as tmpp, \
            tc.tile_pool(name="ptp", bufs=6) as ptp, \
            tc.tile_pool(name="plp", bufs=2) as plp, \
            tc.tile_pool(name="smallp", bufs=4) as smallp, \
            tc.tile_pool(name="yp", bufs=6) as yp, \
            tc.tile_pool(name="psmm", bufs=2, space="PSUM") as psmm, \
            tc.tile_pool(name="pss", bufs=2, space="PSUM") as pss, \
            tc.tile_pool(name="pstr", bufs=2, space="PSUM") as pstr, \
            tc.tile_pool(name="pso", bufs=2, space="PSUM") as pso:

        # ---- constants / weights ----
        w_sb = const.tile([P, D // P, F3], fmm, tag="w")
        nc.sync.dma_start(w_sb[:, :, :], wq_r)
        wo_sb = const.tile([FD, D], fmm, tag="wo")
        nc.sync.dma_start(wo_sb[:, :], woT[:, :])
        caus_sb = const.tile([P, P], f32, tag="caus")
        nc.sync.dma_start(caus_sb[:, :], caus[:, :])
        # 0/1 multiplicative causal mask in matmul dtype (applied post-exp)
        caus01 = const.tile([P, P], fmm, tag="caus01")
        nc.vector.tensor_scalar(caus01, caus_sb, -0.5, None, ALU.is_ge)
        madd_sb = const.tile([P, B * NB], f32, tag="madd")
        nc.sync.dma_start(madd_sb[:, :], madd[:, :])
        freq_sb = const.tile([P, NB, DH], f32, tag="freq")
        nc.sync.dma_start(freq_sb[:, :, :], freq_r)
        ident_f32 = const.tile([P, P], f32, tag="ident_f32")
        make_identity(nc, ident_f32)
        ident = const.tile([P, P], fmm, tag="ident")
        nc.vector.tensor_copy(ident, ident_f32)
        onecol = const.tile([P, 1], f32, tag="onecol")
        nc.gpsimd.memset(onecol, 1.0)

        # cos = sin(wrap(freq + pi/2)); sin_signed: negated at even dh positions.
        # Scalar-engine Sin needs inputs in [-pi, pi]; add_range_wrap handles
        # the shift + one-period wrap (valid for |freq| < 3*pi - shift).
        PI = float(np.pi)

        def range_wrap(out, xs, scratch):
            # out = xs - 2*pi * ((xs > pi) - (xs < -pi)) : one-period wrap
            g = scratch.tile([P, NB, DH], f32, tag="wrap_g")
            lo = scratch.tile([P, NB, DH], f32, tag="wrap_l")
            del scratch
            nc.vector.tensor_scalar(g, xs, PI, None, ALU.is_gt)
            nc.vector.tensor_scalar(lo, xs, -PI, None, ALU.is_lt)
            nc.vector.tensor_tensor(g, g, lo, ALU.subtract)
            nc.vector.scalar_tensor_tensor(out, g, -2 * PI, xs, ALU.mult, ALU.add)

        wrap_s = const.tile([P, NB, DH], f32, tag="wrap_s")
        range_wrap(wrap_s, freq_sb, const)
        wrap_c = const.tile([P, NB, DH], f32, tag="wrap_c")
        shifted = const.tile([P, NB, DH], f32, tag="shifted")
        nc.vector.tensor_scalar(shifted, freq_sb, PI / 2, None, ALU.add)
        range_wrap(wrap_c, shifted, const)
        cos_sb = const.tile([P, NB, DH], f32, tag="cos")
        nc.scalar.activation(cos_sb, wrap_c, AF.Sin)
        sins_sb = const.tile([P, NB, DH], f32, tag="sins")
        nc.scalar.activation(sins_sb[:, :, 0::2], wrap_s[:, :, 0::2], AF.Sin,
                             scale=-1.0)
        nc.scalar.activation(sins_sb[:, :, 1::2], wrap_s[:, :, 1::2], AF.Sin,
                             scale=1.0)

        for b in range(B):
            # ================= Phase B: QKV projection + rotary ============
            qkB = qkp.tile([P, NB, 2 * FD], fmm, tag="qkB")     # q01|k01
            vfB = vfp.tile([P, NB, HPC * (DH + 1)], fmm, tag="vfB")
            # ones column at position 64 of each head's 65-wide group
            # (copy-cast from an f32 const: memset can't encode float32r)
            nc.vector.tensor_copy(vfB[:, :, DH::DH + 1],
                                  onecol[:, None, :].to_broadcast([P, NB, HPC]))

            for c in range(N // 512):                # 512-token chunks
                x_sb = xp.tile([P, D // P, 512], fmm, tag="x")
                tok0 = b * N + c * 512
                nc.sync.dma_start(x_sb[:, :, :], xT_r[:, :, tok0:tok0 + 512])
                for tb in range(4):
                    t = c * 4 + tb                   # token-block in batch
                    qkv_ps = psmm.tile([P, F3], f32, tag="mm")
                    for ko in range(D // P):
                        nc.tensor.matmul(
                            qkv_ps,
                            mm(x_sb[:, ko, tb * P:(tb + 1) * P]),
                            mm(w_sb[:, ko, :]),
                            start=(ko == 0), stop=(ko == D // P - 1),
                        )
                    ps_g = qkv_ps.rearrange("p (g d) -> p g d", g=6)
                    # pass 1: tmp = pairswap(qkv) * sin_signed
                    tmp = tmpp.tile([P, F3], f32, tag="tmp")
                    tmp_g = tmp.rearrange("p (g d) -> p g d", g=6)
                    se = sins_sb[:, t, 0::2][:, None, :].to_broadcast([P, 6, DH // 2])
                    so = sins_sb[:, t, 1::2][:, None, :].to_broadcast([P, 6, DH // 2])
                    nc.vector.tensor_tensor(tmp_g[:, :, 0::2], ps_g[:, :, 1::2], se, ALU.mult)
                    nc.vector.tensor_tensor(tmp_g[:, :, 1::2], ps_g[:, :, 0::2], so, ALU.mult)
                    # pass 2: cosq = qkv * cos
                    cosq = tmpp.tile([P, F3], f32, tag="cosq")
                    cosq_g = cosq.rearrange("p (g d) -> p g d", g=6)
                    cb = cos_sb[:, t, :][:, None, :].to_broadcast([P, 6, DH])
                    nc.vector.tensor_tensor(cosq_g, ps_g, cb, ALU.mult)
                    # pass 3: rotated = tmp + cosq (q,k -> qkB; v -> vfB)
                    nc.gpsimd.tensor_tensor(qkB[:, t, :], tmp[:, 0:2 * FD],
                                            cosq[:, 0:2 * FD], ALU.add)
                    vf_v = vfB[:, t, :].rearrange("p (h c) -> p h c", h=HPC)[:, :, 0:DH]
                    tmp_v = tmp[:, 2 * FD:F3].rearrange("p (h d) -> p h d", h=HPC)
                    cos_v = cosq[:, 2 * FD:F3].rearrange("p (h d) -> p h d", h=HPC)
                    nc.gpsimd.tensor_tensor(vf_v, tmp_v, cos_v, ALU.add)

            # ================= Phase C: transpose q,k to [dh, tok] =========
            # Q is split into per-head tiles zero-padded to 128 contraction
            # rows, so the S matmul can use the full 2-head K block as lhsT
            # with K=128 (K=64 matmuls run the PE at half rate).
            QT0 = qtp.tile([P, N], fmm, tag="QT0")
            QT1 = qtp.tile([P, N], fmm, tag="QT1")
            KT = qtp.tile([P, N], fmm, tag="KT")
            nc.gpsimd.memset(QT0[DH:P, :], 0.0)
            nc.gpsimd.memset(QT1[0:DH, :], 0.0)
            for t in range(NB):
                for which in (0, 1):
                    src = qkB[:, t, which * FD:(which + 1) * FD]
                    tr_ps = pstr.tile([P, P], fmm, tag="tr")
                    nc.tensor.transpose(tr_ps, src, ident)
                    if which == 0:
                        nc.vector.tensor_copy(QT0[0:DH, t * P:(t + 1) * P],
                                              tr_ps[0:DH, :])
                        nc.scalar.copy(QT1[DH:P, t * P:(t + 1) * P],
                                       tr_ps[DH:P, :])
                    elif t % 2 == 0:
                        nc.vector.tensor_copy(KT[:, t * P:(t + 1) * P], tr_ps)
                    else:
                        nc.scalar.copy(KT[:, t * P:(t + 1) * P], tr_ps)

            # ================= Phase D: attention ==========================
            # Software-pipelined: the S matmul for chunk i+1 is emitted before
            # exp/O of chunk i so the PE never stalls on the ScalarE exp.
            O_tiles = {}
            PL_tiles = {}
            QTs = (QT0, QT1)

            def emit_proj(qc, b=b):
                PLq = PL_tiles.pop(qc)
                for tb in range(4):
                    t = qc * 4 + tb
                    y_sb = yp.tile([P, D], f32, tag="ysb")
                    for dc in range(2):
                        y_ps = psmm.tile([P, 512], f32, tag="mm", name=f"yps{dc}")
                        nc.tensor.matmul(y_ps,
                                         mm(PLq[:, tb * P:(tb + 1) * P]),
                                         mm(wo_sb[:, dc * 512:(dc + 1) * 512]),
                                         start=True, stop=True)
                        if dc == 0:
                            nc.vector.tensor_copy(y_sb[:, 0:512], y_ps)
                        else:
                            nc.scalar.copy(y_sb[:, 512:1024], y_ps)
                    r0 = b * N + t * P
                    nc.sync.dma_start(y[r0:r0 + P, :], y_sb)

            def emit_S(qc, kb, h):
                qs = max(kb * P, 512 * qc)
                off = qs - 512 * qc
                w = 512 - off
                S_t = pss.tile([P, 512], f32, tag="s")
                nc.tensor.matmul(S_t[:, :w], mm(KT[:, kb * P:(kb + 1) * P]),
                                 mm(QTs[h][:, qs:qs + w]), start=True, stop=True)
                return (qc, kb, h, off, w, S_t)

            def emit_expO(qc, kb, h, off, w, S_t, b=b):
                pt = ptp.tile([P, 512], fmm, tag="pt")
                col = b * NB + kb
                nc.scalar.activation(pt[:, :w], S_t[:, :w], AF.Exp,
                                     bias=madd_sb[:, col:col + 1],
                                     scale=SCALE)
                if kb >= 4 * qc:  # chunk starts at the diagonal block
                    nc.vector.tensor_tensor(pt[:, 0:P], pt[:, 0:P], caus01,
                                            ALU.mult)
                if kb == 0:
                    O_tiles[(h, qc)] = pso.tile([DH + 1, 512], f32, tag="o",
                                                name=f"O_{h}_{qc}")
                O_ps = O_tiles[(h, qc)]
                nc.tensor.matmul(
                    O_ps[:, off:512],
                    mm(vfB[:, kb, h * (DH + 1):(h + 1) * (DH + 1)]),
                    mm(pt[:, :w]),
                    start=(kb == 0), stop=(kb == 4 * qc + 3),
                )
                if kb == 4 * qc + 3:
                    # normalize: PL[h] = O^T * broadcast(1/Z).  1/Z via a fold
                    # to [128, 4] so the DVE reciprocal runs on all lanes.
                    zrow = smallp.tile([1, 512], f32, tag="zrow")
                    nc.vector.tensor_copy(zrow, O_ps[DH:DH + 1, :])
                    zf = smallp.tile([P, 4], f32, tag="zf")
                    nc.sync.dma_start(
                        zf[:, :], zrow.rearrange("o (p c) -> o p c", p=P))
                    rf = smallp.tile([P, 4], f32, tag="rf")
                    nc.vector.reciprocal(rf, zf)
                    rrow = smallp.tile([1, 512], f32, tag="rrow")
                    nc.sync.dma_start(
                        rrow.rearrange("o (p c) -> o p c", p=P), rf[:, :])
                    rb = smallp.tile([DH, 512], f32, tag="rb")
                    nc.gpsimd.partition_broadcast(rb, rrow)
                    if qc not in PL_tiles:
                        PL_tiles[qc] = plp.tile([P, 512], fmm, tag="PL",
                                                name=f"PL_{qc}")
                    nc.vector.tensor_tensor(
                        PL_tiles[qc][h * DH:(h + 1) * DH, :],
                        O_ps[0:DH, :], rb, ALU.mult)
                    if h == HPC - 1:
                        emit_proj(qc)

            units = [(qc, kb, h) for qc in range(QC)
                     for kb in range(4 * qc + 4) for h in range(HPC)]
            pend = None
            for u in units:
                cur = emit_S(*u)
                if pend is not None:
                    emit_expO(*pend)
                pend = cur
            emit_expO(*pend)


    nc.compile()
    return nc


def prep_inputs(x, mask, rotary_pos_emb, W_qkv, W_out, dt_mode="f16"):
    """Host-side shard prep: per-core input dicts (layout only + mask encode)."""
    np_mm = np.float16 if dt_mode == "f16" else np.float32
    x = np.asarray(x, dtype=np.float32)
    W_qkv = np.asarray(W_qkv, dtype=np.float32)
    W_out = np.asarray(W_out, dtype=np.float32)
    rope = np.asarray(rotary_pos_emb, dtype=np.float32)
    mask = np.asarray(mask)

    xT = np.ascontiguousarray(x.reshape(NT, D).T.astype(np_mm))
    madd = np.where(mask, np.float32(0.0), np.float32(NEG)).astype(np.float32)
    madd_dev = np.ascontiguousarray(
        madd.reshape(B, NB, P).transpose(2, 0, 1).reshape(P, B * NB))
    kidx = np.arange(P)[:, None]
    qidx = np.arange(P)[None, :]
    caus = np.where(qidx >= kidx, np.float32(0.0), np.float32(NEG)).astype(np.float32)
    freq = np.ascontiguousarray(rope[-N:, :])

    in_maps = []
    for c in range(NCORES):
        rows = []
        for tsel in range(3):                      # q, k, v row blocks
            for h in (HPC * c, HPC * c + 1):
                o = tsel * H * DH + h * DH
                rows.append(W_qkv[o:o + DH, :])
        wqkvT = np.ascontiguousarray(np.concatenate(rows, axis=0).T.astype(np_mm))
        woT = np.ascontiguousarray(W_out[:, FD * c:FD * (c + 1)].T.astype(np_mm))
        in_maps.append({
            "xT": xT, "wqkvT": wqkvT, "woT": woT,
            "freq": freq, "madd": madd_dev, "caus": caus,
        })
    return in_maps


def _ensure_ntff_hook():
    """Install antenv.axon_hooks + the ctypes NTFF profile hook if the image
    lacks them (needed only for trace=True timing runs, not for kernel())."""
    import types
    try:
        from antenv.axon_hooks import get_axon_ntff_profile_hook  # noqa: F401
        return
    except ImportError:
        pass
    try:
        import antenv
        mod = types.ModuleType("antenv.axon_hooks")
        _state = {"hook": None}

        def set_axon_ntff_profile_hook(h):
            _state["hook"] = h

        def get_axon_ntff_profile_hook():
            return _state["hook"]

        mod.set_axon_ntff_profile_hook = set_axon_ntff_profile_hook
        mod.get_axon_ntff_profile_hook = get_axon_ntff_profile_hook
        sys.modules["antenv.axon_hooks"] = mod
        antenv.axon_hooks = mod
        from trn_agent_boot.trn_boot import _ntff_profile_via_ctypes
        hook = _ntff_profile_via_ctypes("/opt/axon/libaxon_pjrt.so")
        if hook is not None:
            set_axon_ntff_profile_hook(hook)
    except Exception as e:  # degrade to untimed runs
        print(f"ntff hook install failed: {e!r}", file=sys.stderr)


_NC_CACHE = {}


def _get_nc(dt_mode="f16"):
    if dt_mode not in _NC_CACHE:
        _NC_CACHE[dt_mode] = build_nc(dt_mode=dt_mode)
    return _NC_CACHE[dt_mode]


def run_cores(in_maps, trace=False, dt_mode="f16"):
    if trace:
        _ensure_ntff_hook()
    nc = _get_nc(dt_mode)
    res = run_bass_kernel_spmd(
        nc, in_maps, core_ids=list(range(NCORES)), trace=trace,
        trace_cores=list(range(NCORES)) if trace else None,
    )
    return res


DT_MODE = "f16"


def kernel(x, mask, rotary_pos_emb, W_qkv, W_out, b_out):
    in_maps = prep_inputs(x, mask, rotary_pos_emb, W_qkv, W_out, dt_mode=DT_MODE)
    res = run_cores(in_maps, trace=False, dt_mode=DT_MODE)
    y = np.zeros((NT, D), dtype=np.float32)
    for r in res.results:
        y += r["y"]
    y += np.asarray(b_out, dtype=np.float32)[None, :]
    return y.reshape(B, N, D)
